# revision 42
# baseline (speedup 1.0000x reference)
"""Multi-head causal attention (SEQ=4096, D=1024, H=16, DK=64) on 8 TRN2
NeuronCores, tensor-parallel over heads (2 heads/core). Self-contained.

Per-core pipeline (v2):
  1. Projections: Qh^T/Kh^T/Vh^T = W.T @ X^T in fp32 (X^T pre-transposed on
     host, 1/sqrt(dk) folded into W_Q host-side). PSUM results copied to f32r
     tiles: per-head qT/kT (rows 0-63 + aux rows) and 2-head packed
     qhT2/khT2 (h0 -> partitions 0-63, h1 -> 64-127) for row-tiled stats.
  2. Stats pass: S = Qh^T.T @ Kh^T, single f32r matmul per tile, the two
     heads' matmuls row-tiled via tile_position (0,0)/(64,0) so they run
     concurrently in disjoint array row-groups. Causal mask via
     identity-matmul of a -1e9 tile. Row-max reduced on DVE, negated, and
     DMA-transposed into row 64 of the per-head qT tile.
  3. S^T pass: single f32r matmul S^T[kc,q] = [Kh;1].T @ [Qh;-m] (the max
     subtraction rides the contraction as the 65th row). f32r operand
     rounding gives score error ~5e-2 abs (on a +-3500 range), which maps to
     ~2e-3 output rel err (empirically calibrated) - far inside the 2e-2
     gate. Mask via identity matmul, ACT exp -> P^T. AV in f32r:
     ones-augmented Vh gives l = sum(exp) as row 64 of the PSUM accumulator.
  4. 1/l per qb (DVE reciprocal), Y_partial = C^T.T @ W_O_rows with the two
     heads' matmuls row-tiled, scaled by 1/l on ACT, summed on DVE, DMA out.
     Host sums the 8 per-core partials.
"""

import os
import sys

sys.path.insert(0, "/opt/trn_rl_repo")

import numpy as np
import ml_dtypes

import concourse.bass as bass
import concourse.mybir as mybir
import concourse.tile as tile
from concourse.bass_utils import run_bass_kernel_spmd
from concourse.masks import make_identity

P = 128
S = 4096
D = 1024
DK = 64
NH = 2  # heads per core
NCORES = 8
NEG = -1.0e9
F32 = mybir.dt.float32
F32R = mybir.dt.float32r
BF16 = mybir.dt.bfloat16
EXP = mybir.ActivationFunctionType.Exp

_ctr = [0]


def _split_waits(nc, max_waits=1):
    """walrus rejects >1 sem-wait per instruction; move extras onto
    preceding same-engine NOPs (engine streams are program-ordered)."""
    for f in nc.m.functions:
        for bb in f.blocks:
            insts = bb.instructions
            new = []
            changed = False
            for inst in insts:
                si = inst.sync_info
                if si is not None and si.on_wait and len(si.on_wait) > max_waits:
                    waits = list(si.on_wait)
                    extra, keep = waits[:-max_waits], waits[-max_waits:]
                    for i in range(0, len(extra), max_waits):
                        _ctr[0] += 1
                        new.append(
                            mybir.InstNoOp(
                                name=f"waitsplit-{_ctr[0]}",
                                engine=inst.engine,
                                ins=[],
                                outs=[],
                                sync_info=mybir.SyncInfo(
                                    on_wait=extra[i : i + max_waits], on_update=[]
                                ),
                            )
                        )
                    inst.sync_info = mybir.SyncInfo(
                        on_wait=keep, on_update=list(si.on_update)
                    )
                    changed = True
                new.append(inst)
            if changed:
                bb.instructions = new


def build(nc: bass.Bass, causal: bool = True):
    repeat = int(os.environ.get("ATTN_REPEAT", "1"))
    NB = S // 512  # 8   512-wide blocks
    QB = S // P  # 32  128-wide q blocks
    DC = D // P  # 8   128-deep contraction chunks

    qThl = nc.dram_tensor("qThl", [D, 2, S], BF16, kind="ExternalInput")
    kThl = nc.dram_tensor("kThl", [D, 2, S], BF16, kind="ExternalInput")
    vT = nc.dram_tensor("vT", [D, S], BF16, kind="ExternalInput")
    wqhi = nc.dram_tensor("wqhi", [D, NH * DK], BF16, kind="ExternalInput")
    wqlo = nc.dram_tensor("wqlo", [D, NH * DK], BF16, kind="ExternalInput")
    wkhi = nc.dram_tensor("wkhi", [D, NH * DK], BF16, kind="ExternalInput")
    wklo = nc.dram_tensor("wklo", [D, NH * DK], BF16, kind="ExternalInput")
    wv = nc.dram_tensor("wv", [D, NH * DK], BF16, kind="ExternalInput")
    wo = nc.dram_tensor("wo", [NH * DK, D], F32R, kind="ExternalInput")
    maskf = nc.dram_tensor("maskf", [P, P], BF16, kind="ExternalInput")
    maskb = nc.dram_tensor("maskb", [P, P], BF16, kind="ExternalInput")
    onesr = nc.dram_tensor("onesr", [1, S], F32R, kind="ExternalInput")
    y0 = nc.dram_tensor("y0", [S, D], BF16, kind="ExternalOutput")

    with tile.TileContext(nc) as tc:
        import contextlib

        ctx = contextlib.ExitStack()
        with ctx:
            const = ctx.enter_context(tc.tile_pool(name="const", bufs=1))
            big = ctx.enter_context(tc.tile_pool(name="big", bufs=1))
            stream = ctx.enter_context(
                tc.tile_pool(name="stream", bufs=int(os.environ.get("ATTN_BSTREAM", "6")))
            )
            pstream = ctx.enter_context(
                tc.tile_pool(name="pstream", bufs=int(os.environ.get("ATTN_BPS", "16")))
            )
            ptp = ctx.enter_context(
                tc.tile_pool(name="ptp", bufs=int(os.environ.get("ATTN_BPT", "3")))
            )
            ypool = ctx.enter_context(
                tc.tile_pool(name="ypool", bufs=int(os.environ.get("ATTN_BY", "2")))
            )
            smalls = ctx.enter_context(
                tc.tile_pool(name="smalls", bufs=int(os.environ.get("ATTN_BSM", "4")))
            )
            bproj = int(os.environ.get("ATTN_BPROJ", "2"))
            bstat = int(os.environ.get("ATTN_BSTAT", "2"))
            bst = int(os.environ.get("ATTN_BST", "2"))
            bmisc = int(os.environ.get("ATTN_BMISC", "2"))
            ps_proj = ctx.enter_context(
                tc.tile_pool(name="ps_proj", bufs=bproj, space="PSUM")
            )
            ps_stat = ctx.enter_context(
                tc.tile_pool(name="ps_stat", bufs=bstat, space="PSUM")
            )
            ps_st = ctx.enter_context(tc.tile_pool(name="ps_st", bufs=bst, space="PSUM"))
            ps_misc = ctx.enter_context(
                tc.tile_pool(name="ps_misc", bufs=bmisc, space="PSUM")
            )

            # ---- constants ----
            ident = const.tile([P, P], F32)
            make_identity(nc, ident[:])
            ident_b = const.tile([P, P], BF16)
            nc.vector.tensor_copy(ident_b[:], ident[:])
            ident_r = const.tile([P, P], F32R)
            nc.vector.tensor_copy(ident_r[:], ident[:])

            wq_hi_sb = const.tile([P, DC, P], BF16, tag="wqh")
            wq_lo_sb = const.tile([P, DC, P], BF16, tag="wql")
            wk_hi_sb = const.tile([P, DC, P], BF16, tag="wkh")
            wk_lo_sb = const.tile([P, DC, P], BF16, tag="wkl")
            wv_sb = const.tile([P, DC, P], BF16, tag="wv")
            nc.sync.dma_start(wq_hi_sb[:], wqhi.rearrange("(o p) m -> p o m", p=P))
            nc.sync.dma_start(wq_lo_sb[:], wqlo.rearrange("(o p) m -> p o m", p=P))
            nc.sync.dma_start(wk_hi_sb[:], wkhi.rearrange("(o p) m -> p o m", p=P))
            nc.sync.dma_start(wk_lo_sb[:], wklo.rearrange("(o p) m -> p o m", p=P))
            nc.sync.dma_start(wv_sb[:], wv.rearrange("(o p) m -> p o m", p=P))
            wo_sb = const.tile([P, D], F32R, tag="wo")
            nc.sync.dma_start(wo_sb[:], wo[:])

            mf_sb = const.tile([P, P], BF16, tag="mf")
            mb_sb = const.tile([P, P], BF16, tag="mb")
            nc.sync.dma_start(mf_sb[:], maskf[:])
            nc.sync.dma_start(mb_sb[:], maskb[:])

            # ---- persistent activations ----
            # qTa[h]: rows 0-63 = Qh^T (f32r), row 64 = -m (written per qb by
            # stats), row 96 = l stash. kTa[h]: rows 0-63 = Kh^T, row 64 = ones.
            # qhT2/khT2: 2-head packed (h0 -> 0-63, h1 -> 64-127) for the
            # row-tiled stats matmuls.
            qTa = [big.tile([P, S], F32R, tag=f"qTa{h}", name=f"qTa{h}") for h in range(NH)]
            kTa = [big.tile([P, S], F32R, tag=f"kTa{h}", name=f"kTa{h}") for h in range(NH)]
            qhT2 = big.tile([P, S], F32R, tag="qhT2", name="qhT2")
            khT2 = big.tile([P, S], F32R, tag="khT2", name="khT2")
            vh = [big.tile([P, QB, DK + 1], F32R, tag=f"vh{h}", name=f"vh{h}") for h in range(NH)]
            ct = big.tile([P, S], F32R, tag="ct")
            mcol = [big.tile([P, QB], F32R, tag=f"mcol{h}", name=f"mcol{h}") for h in range(NH)]
            lcol = [big.tile([P, QB], F32R, tag=f"lcol{h}", name=f"lcol{h}") for h in range(NH)]
            rcol = [big.tile([P, QB], F32, tag=f"rcol{h}", name=f"rcol{h}") for h in range(NH)]

            ones_qb = const.tile([P, QB], F32, tag="ones_qb")
            nc.any.memset(ones_qb[:], 1.0)
            for h in range(NH):
                nc.sync.dma_start(kTa[h][DK : DK + 1, :], onesr[:])  # ones row
                nc.vector.tensor_copy(vh[h][:, :, DK], ones_qb[:])  # ones col

            for _rep in range(repeat):
                # ---- Q/K projection block emitters ----
                # fp32 accuracy via bf16 hi/lo pairs (host-split): X@W =
                # Xhi@Whi + Xhi@Wlo + Xlo@Whi (lo*lo term negligible), each
                # a 1-cycle/row bf16 matmul vs fp32's 4 cycles/row.
                def proj_emit(t_idx, nb):
                    xhl, w_hi, w_lo = [
                        (qThl, wq_hi_sb, wq_lo_sb),
                        (kThl, wk_hi_sb, wk_lo_sb),
                    ][t_idx]
                    ps = ps_proj.tile([P, 512], F32, tag="proj", name="ps")
                    for dc in range(DC):
                        xt = stream.tile([P, 2, 512], BF16, tag="xin", name="xt")
                        dsl = slice(dc * P, (dc + 1) * P)
                        nsl = slice(nb * 512, (nb + 1) * 512)
                        nc.sync.dma_start(xt[:], xhl[dsl, :, nsl])
                        nc.tensor.matmul(
                            ps[:], w_hi[:, dc, :], xt[:, 0, :],
                            start=(dc == 0), stop=False,
                        )
                        nc.tensor.matmul(
                            ps[:], w_lo[:, dc, :], xt[:, 0, :],
                            start=False, stop=False,
                        )
                        nc.tensor.matmul(
                            ps[:], w_hi[:, dc, :], xt[:, 1, :],
                            start=False, stop=(dc == DC - 1),
                        )
                    sl = slice(nb * 512, (nb + 1) * 512)
                    pk2 = qhT2 if t_idx == 0 else khT2
                    dst = qTa if t_idx == 0 else kTa
                    nc.scalar.copy(pk2[:, sl], ps[:])
                    for h in range(NH):
                        nc.scalar.copy(
                            dst[h][0:DK, sl], ps[h * DK : (h + 1) * DK, :]
                        )

                def vproj_emit(nb):
                    ps = ps_proj.tile([P, 512], F32, tag="proj", name="ps")
                    for dc in range(DC):
                        xt = stream.tile([P, 512], BF16, tag="xin", name="xtv")
                        nc.sync.dma_start(
                            xt[:],
                            vT[dc * P : (dc + 1) * P, nb * 512 : (nb + 1) * 512],
                        )
                        nc.tensor.matmul(
                            ps[:],
                            wv_sb[:, dc, :],
                            xt[:],
                            start=(dc == 0),
                            stop=(dc == DC - 1),
                        )
                    vtmp = stream.tile([P, 512], F32R, tag="xin", name="vtmp")
                    nc.vector.tensor_copy(vtmp[:], ps[:])
                    for h in range(NH):
                        pst = ps_misc.tile([P, 512], F32R, tag="misc", name="pst")
                        for j in range(4):
                            nc.tensor.transpose(
                                pst[0:P, j * DK : (j + 1) * DK],
                                vtmp[h * DK : (h + 1) * DK, j * P : (j + 1) * P],
                                ident_r[h * DK : (h + 1) * DK, h * DK : (h + 1) * DK],
                            )
                        nc.vector.tensor_copy(
                            vh[h][:, nb * 4 : nb * 4 + 4, 0:DK],
                            pst[:, 0 : 4 * DK].rearrange("p (j d) -> p j d", j=4),
                        )

                # ---- stats: row max via row-tiled 2-head matmuls ----
                # Emitted as fine-grained filler units popped between S^T kc
                # steps so the DVE reduces drain under S^T compute and the
                # two ps_stat banks recycle without stalling PE.
                ID = mybir.ActivationFunctionType.Identity
                from collections import deque

                stats_q = deque()  # (batch_nb, emit)
                wo_q = deque()
                proj_q = deque()  # (block_nb, emit) dc-chunk fillers
                mparts_of = {}
                acc_of = {}

                def proj_unit(t_idx, nb, dc, xt):
                    def emit():
                        key = (t_idx, nb)
                        w_hi, w_lo = [
                            (wq_hi_sb, wq_lo_sb),
                            (wk_hi_sb, wk_lo_sb),
                        ][t_idx]
                        if dc == 0:
                            acc_of[key] = ps_proj.tile(
                                [P, 512], F32, tag="proj", name="ps"
                            )
                        ps = acc_of[key]
                        nc.tensor.matmul(
                            ps[:], w_hi[:, dc, :], xt[:, 0, :],
                            start=(dc == 0), stop=False,
                        )
                        nc.tensor.matmul(
                            ps[:], w_lo[:, dc, :], xt[:, 0, :],
                            start=False, stop=False,
                        )
                        nc.tensor.matmul(
                            ps[:], w_hi[:, dc, :], xt[:, 1, :],
                            start=False, stop=(dc == DC - 1),
                        )
                        if dc == DC - 1:
                            del acc_of[key]
                            sl = slice(nb * 512, (nb + 1) * 512)
                            pk2 = qhT2 if t_idx == 0 else khT2
                            dst = qTa if t_idx == 0 else kTa
                            nc.scalar.copy(pk2[:, sl], ps[:])
                            for h in range(NH):
                                nc.scalar.copy(
                                    dst[h][0:DK, sl],
                                    ps[h * DK : (h + 1) * DK, :],
                                )

                    return emit

                def queue_proj(nb):
                    # DMAs are issued at queue time (one iteration ahead of
                    # consumption) so their latency hides under the S^T steps
                    for t_idx in (1, 0):  # K then Q
                        xhl = [qThl, kThl][t_idx]
                        for dc in range(DC):
                            xt = pstream.tile([P, 2, 512], BF16, tag="pxin", name="xt")
                            nc.sync.dma_start(
                                xt[:],
                                xhl[dc * P : (dc + 1) * P, :, nb * 512 : (nb + 1) * 512],
                            )
                            proj_q.append((nb, proj_unit(t_idx, nb, dc, xt)))

                def drain_proj(max_nb):
                    while proj_q and proj_q[0][0] <= max_nb:
                        proj_q.popleft()[1]()

                def pop_filler():
                    # proj before stats: stats units read the proj copies, so
                    # their matmuls must trail them in the PE stream
                    if proj_q:
                        proj_q.popleft()[1]()
                        return True
                    if stats_q:
                        stats_q.popleft()[1]()
                        return True
                    if wo_q:
                        wo_q.popleft()()
                        return True
                    return False

                def stats_unit(qb, kc):
                    def emit():
                        kmax = qb // 4 + 1 if causal else NB
                        if kc == 0:
                            mparts_of[qb] = [
                                smalls.tile([P, NB], F32, tag="mpart", name=f"mpart{h}")
                                for h in range(NH)
                            ]
                        mparts = mparts_of[qb]
                        diag = causal and (kc == qb // 4)
                        nv = (qb % 4) * P + P if diag else 512
                        pss = []
                        for h in range(NH):
                            ps = ps_stat.tile([P, 512], F32, tag="stat", name="ps_stat")
                            nc.tensor.matmul(
                                ps[:, 0:nv],
                                qhT2[h * DK : (h + 1) * DK, qb * P : (qb + 1) * P],
                                khT2[h * DK : (h + 1) * DK, kc * 512 : kc * 512 + nv],
                                start=True,
                                stop=not diag,
                            )
                            pss.append(ps)
                        for h in range(NH):
                            if diag:
                                nc.tensor.matmul(
                                    pss[h][:, nv - P : nv],
                                    ident_b[:],
                                    mb_sb[:],
                                    start=False,
                                    stop=True,
                                )
                            nc.vector.reduce_max(
                                mparts[h][:, kc : kc + 1],
                                pss[h][:, 0:nv],
                                axis=mybir.AxisListType.X,
                            )
                        if kc == kmax - 1:
                            del mparts_of[qb]
                            for h in range(NH):
                                nc.vector.tensor_reduce(
                                    mcol[h][:, qb : qb + 1],
                                    mparts[h][:, 0:kmax],
                                    axis=mybir.AxisListType.X,
                                    op=mybir.AluOpType.max,
                                    negate=True,
                                )
                                nc.sync.dma_start(
                                    qTa[h][DK : DK + 1, qb * P : (qb + 1) * P],
                                    mcol[h][:, qb : qb + 1],
                                )

                    return emit

                def queue_stats(batch):
                    for qb in range(4 * batch, 4 * batch + 4):
                        kmax = qb // 4 + 1 if causal else NB
                        for kc in range(kmax):
                            stats_q.append((batch, stats_unit(qb, kc)))

                def drain_stats(max_batch):
                    # force-complete stats units for batches <= max_batch
                    # (gates the S^T pass that reads their m rows)
                    while stats_q and stats_q[0][0] <= max_batch:
                        stats_q.popleft()[1]()

                # ---- S^T + exp + AV ----
                def st3_emit(h, nb):
                    nkc = 4 * (nb + 1) if causal else QB
                    po = ps_misc.tile([P, 512], F32, tag="misc", name="po")
                    pss = {}

                    def s_mm(kc):
                        ps = ps_st.tile([P, 512], F32, tag="st", name="ps_st")
                        diag = causal and (kc >= 4 * nb)
                        o = kc - 4 * nb if diag else 0
                        qoff = o * P
                        nv = 512 - qoff
                        kslice = slice(kc * P, (kc + 1) * P)
                        qslice = slice(nb * 512 + qoff, (nb + 1) * 512)
                        nc.tensor.matmul(
                            ps[:, 0:nv],
                            kTa[h][0 : DK + 1, kslice],
                            qTa[h][0 : DK + 1, qslice],
                            start=True,
                            stop=not diag,
                        )
                        if diag:
                            nc.tensor.matmul(
                                ps[:, 0:P],
                                ident_b[:],
                                mf_sb[:],
                                start=False,
                                stop=True,
                            )
                        pss[kc] = (ps, qoff, nv)

                    s_mm(0)
                    for kc in range(nkc):
                        if kc + 1 < nkc:
                            s_mm(kc + 1)
                        ps, qoff, nv = pss.pop(kc)
                        pt = ptp.tile([P, 512], F32R, tag="pt", name="pt")
                        nc.scalar.activation(pt[:, 0:nv], ps[:, 0:nv], EXP)
                        nc.tensor.matmul(
                            po[0 : DK + 1, qoff:512],
                            vh[h][:, kc, :],
                            pt[:, 0:nv],
                            start=(kc == 0),
                            stop=(kc == nkc - 1),
                        )
                        pop_filler()
                        if len(proj_q) > 8:
                            pop_filler()
                    nc.scalar.copy(
                        ct[h * DK : (h + 1) * DK, nb * 512 : (nb + 1) * 512],
                        po[0:DK, :],
                    )
                    nc.vector.tensor_copy(
                        qTa[h][96:97, nb * 512 : (nb + 1) * 512],
                        po[DK : DK + 1, :],
                    )
                    for j in range(4):
                        qb = nb * 4 + j
                        nc.sync.dma_start(
                            lcol[h][:, qb : qb + 1],
                            qTa[h][96:97, qb * P : (qb + 1) * P],
                        )
                    nc.vector.reciprocal(
                        rcol[h][:, nb * 4 : nb * 4 + 4],
                        lcol[h][:, nb * 4 : nb * 4 + 4],
                    )

                def wo_unit(qc, eb):
                    def emit():
                        ysb0 = ypool.tile([P, 512], F32, tag="ysb0", name="ysb0")
                        ysb1 = ypool.tile([P, 512], F32, tag="ysb1", name="ysb1")
                        psys = []
                        for h in range(NH):
                            psy = ps_proj.tile([P, 512], F32, tag="proj", name="psy")
                            nc.tensor.matmul(
                                psy[:],
                                ct[h * DK : (h + 1) * DK, qc * P : (qc + 1) * P],
                                wo_sb[h * DK : (h + 1) * DK, eb * 512 : (eb + 1) * 512],
                                start=True,
                                stop=True,
                            )
                            psys.append(psy)
                        for h, ysb in ((0, ysb0), (1, ysb1)):
                            nc.scalar.activation(
                                ysb[:], psys[h][:], ID, scale=rcol[h][:, qc : qc + 1]
                            )
                        ysbo = ypool.tile([P, 512], BF16, tag="ysbo", name="ysbo")
                        nc.gpsimd.tensor_add(out=ysbo[:], in0=ysb0[:], in1=ysb1[:])
                        nc.sync.dma_start(
                            y0[qc * P : (qc + 1) * P, eb * 512 : (eb + 1) * 512],
                            ysbo[:],
                        )

                    return emit

                def queue_wo(qcs):
                    for qc in qcs:
                        for eb in range(2):
                            wo_q.append(wo_unit(qc, eb))

                # Fully fused (default): block 0 projected directly, every
                # later K/Q projection block enters as dc-chunk fillers popped
                # between S^T kc steps (priority proj > stats > wo), so PE
                # gets proj work while ACT paces exp and DVE drains stats
                # reduces. ATTN_FUSEPROJ=0 projects everything upfront.
                fuse_proj = bool(int(os.environ.get("ATTN_FUSEPROJ", "0")))
                if fuse_proj:
                    proj_emit(1, 0)  # K block 0
                    proj_emit(0, 0)  # Q block 0
                else:
                    for nb in range(NB):
                        proj_emit(1, nb)
                    for nb in range(NB):
                        proj_emit(0, nb)
                queue_stats(0)
                drain_stats(0)
                for nb in range(NB):
                    if nb + 1 < NB:
                        if fuse_proj:
                            queue_proj(nb + 1)
                        queue_stats(nb + 1)
                    if nb >= 1:
                        queue_wo(range((nb - 1) * 4, nb * 4))
                    vproj_emit(nb)
                    drain_proj(nb)
                    drain_stats(nb)
                    st3_emit(0, nb)
                    st3_emit(1, nb)
                queue_wo(range((NB - 1) * 4, NB * 4))
                while proj_q or stats_q or wo_q:
                    pop_filler()

    _split_waits(nc)
    return nc


_cache = {}


def _get_nc(causal: bool):
    if causal not in _cache:
        nc = bass.Bass(trn_type="TRN2")
        build(nc, causal=causal)
        _cache[causal] = nc
    return _cache[causal]


def _host_masks():
    p = np.arange(P)[:, None]
    j = np.arange(P)[None, :]
    # S^T diag tile [kc, q]: nonzero only in the first 128 q-cols: p > j
    maskf = np.where(p > j, NEG, 0.0).astype(ml_dtypes.bfloat16)
    # stats diag tile [q, kc]: nonzero only in the last 128 kc-cols: j > p
    maskb = np.where(j > p, NEG, 0.0).astype(ml_dtypes.bfloat16)
    return maskf, maskb


def make_in_maps(np_inputs):
    Q = np.asarray(np_inputs["Q"], dtype=np.float32)
    K = np.asarray(np_inputs["K"], dtype=np.float32)
    V = np.asarray(np_inputs["V"], dtype=np.float32)
    W_Q = np.asarray(np_inputs["W_Q"], dtype=np.float32)
    W_K = np.asarray(np_inputs["W_K"], dtype=np.float32)
    W_V = np.asarray(np_inputs["W_V"], dtype=np.float32)
    W_O = np.asarray(np_inputs["W_O"], dtype=np.float32)

    def bf16_pair(x):
        hi = x.astype(ml_dtypes.bfloat16)
        lo = (x - hi.astype(np.float32)).astype(ml_dtypes.bfloat16)
        return np.ascontiguousarray(hi), np.ascontiguousarray(lo)

    qThl = np.ascontiguousarray(np.stack(bf16_pair(Q.T), axis=1))
    kThl = np.ascontiguousarray(np.stack(bf16_pair(K.T), axis=1))
    vTh = np.ascontiguousarray(V.T.astype(ml_dtypes.bfloat16))
    maskf, maskb = _host_masks()
    ones_row = np.ones((1, S), dtype=np.float32)

    scale = np.float32(1.0 / np.sqrt(DK))
    in_maps = []
    for c in range(NCORES):
        h0, h1 = 2 * c, 2 * c + 1
        wq2 = np.ascontiguousarray(
            np.concatenate([W_Q[h0] * scale, W_Q[h1] * scale], axis=1)
        ).astype(np.float32)
        wk2 = np.ascontiguousarray(
            np.concatenate([W_K[h0], W_K[h1]], axis=1)
        ).astype(np.float32)
        wqhi, wqlo = bf16_pair(wq2)
        wkhi, wklo = bf16_pair(wk2)
        wv2 = np.ascontiguousarray(
            np.concatenate([W_V[h0], W_V[h1]], axis=1).astype(ml_dtypes.bfloat16)
        )
        wo2 = np.ascontiguousarray(W_O[P * c : P * (c + 1), :])
        in_maps.append(
            {
                "qThl": qThl,
                "kThl": kThl,
                "vT": vTh,
                "wqhi": wqhi,
                "wqlo": wqlo,
                "wkhi": wkhi,
                "wklo": wklo,
                "wv": wv2,
                "wo": wo2,
                "maskf": maskf,
                "maskb": maskb,
                "onesr": ones_row,
            }
        )
    return in_maps


LAST_EXEC_NS = None


def kernel(Q, K, V, W_Q, W_K, W_V, W_O, mask):
    global LAST_EXEC_NS
    causal = bool(np.asarray(mask).item())
    nc = _get_nc(causal)
    in_maps = make_in_maps(
        dict(Q=Q, K=K, V=V, W_Q=W_Q, W_K=W_K, W_V=W_V, W_O=W_O)
    )

    trace = bool(int(os.environ.get("ATTN_TRACE", "0")))
    res = run_bass_kernel_spmd(
        nc, in_maps, core_ids=list(range(NCORES)), trace=trace
    )
    LAST_EXEC_NS = res.exec_time_ns

    out = np.zeros((S, D), dtype=np.float32)
    for c in range(NCORES):
        out += res.results[c]["y0"].astype(np.float32)
    return out


# revision 43
# speedup vs baseline: 1.0934x; 1.0934x over previous
"""Multi-head causal attention (SEQ=4096, D=1024, H=16, DK=64) on 8 TRN2
NeuronCores, tensor-parallel over heads (2 heads/core). Self-contained.

Per-core pipeline (v2):
  1. Projections: Qh^T/Kh^T/Vh^T = W.T @ X^T in fp32 (X^T pre-transposed on
     host, 1/sqrt(dk) folded into W_Q host-side). PSUM results copied to f32r
     tiles: per-head qT/kT (rows 0-63 + aux rows) and 2-head packed
     qhT2/khT2 (h0 -> partitions 0-63, h1 -> 64-127) for row-tiled stats.
  2. Stats pass: S = Qh^T.T @ Kh^T, single f32r matmul per tile, the two
     heads' matmuls row-tiled via tile_position (0,0)/(64,0) so they run
     concurrently in disjoint array row-groups. Causal mask via
     identity-matmul of a -1e9 tile. Row-max reduced on DVE, negated, and
     DMA-transposed into row 64 of the per-head qT tile.
  3. S^T pass: single f32r matmul S^T[kc,q] = [Kh;1].T @ [Qh;-m] (the max
     subtraction rides the contraction as the 65th row). f32r operand
     rounding gives score error ~5e-2 abs (on a +-3500 range), which maps to
     ~2e-3 output rel err (empirically calibrated) - far inside the 2e-2
     gate. Mask via identity matmul, ACT exp -> P^T. AV in f32r:
     ones-augmented Vh gives l = sum(exp) as row 64 of the PSUM accumulator.
  4. 1/l per qb (DVE reciprocal), Y_partial = C^T.T @ W_O_rows with the two
     heads' matmuls row-tiled, scaled by 1/l on ACT, summed on DVE, DMA out.
     Host sums the 8 per-core partials.
"""

import os
import sys

sys.path.insert(0, "/opt/trn_rl_repo")

import numpy as np
import ml_dtypes

import concourse.bass as bass
import concourse.mybir as mybir
import concourse.tile as tile
from concourse.bass_utils import run_bass_kernel_spmd
from concourse.masks import make_identity

P = 128
S = 4096
D = 1024
DK = 64
NH = 2  # heads per core
NCORES = 8
NEG = -1.0e9
F32 = mybir.dt.float32
F32R = mybir.dt.float32r
BF16 = mybir.dt.bfloat16
EXP = mybir.ActivationFunctionType.Exp

_ctr = [0]


def _split_waits(nc, max_waits=1):
    """walrus rejects >1 sem-wait per instruction; move extras onto
    preceding same-engine NOPs (engine streams are program-ordered)."""
    for f in nc.m.functions:
        for bb in f.blocks:
            insts = bb.instructions
            new = []
            changed = False
            for inst in insts:
                si = inst.sync_info
                if si is not None and si.on_wait and len(si.on_wait) > max_waits:
                    waits = list(si.on_wait)
                    extra, keep = waits[:-max_waits], waits[-max_waits:]
                    for i in range(0, len(extra), max_waits):
                        _ctr[0] += 1
                        new.append(
                            mybir.InstNoOp(
                                name=f"waitsplit-{_ctr[0]}",
                                engine=inst.engine,
                                ins=[],
                                outs=[],
                                sync_info=mybir.SyncInfo(
                                    on_wait=extra[i : i + max_waits], on_update=[]
                                ),
                            )
                        )
                    inst.sync_info = mybir.SyncInfo(
                        on_wait=keep, on_update=list(si.on_update)
                    )
                    changed = True
                new.append(inst)
            if changed:
                bb.instructions = new


def build(nc: bass.Bass, causal: bool = True):
    repeat = int(os.environ.get("ATTN_REPEAT", "1"))
    # bf16 for the max-stats operands, P/V/C/W_O: bf16 matmuls get a
    # standalone pipelined LDWEIGHTS (f32r must self-load serially on HW)
    bf16p = bool(int(os.environ.get("ATTN_BF16P", "1")))
    DT_P = BF16 if bf16p else F32R
    NB = S // 512  # 8   512-wide blocks
    QB = S // P  # 32  128-wide q blocks
    DC = D // P  # 8   128-deep contraction chunks

    qThl = nc.dram_tensor("qThl", [D, 2, S], BF16, kind="ExternalInput")
    kThl = nc.dram_tensor("kThl", [D, 2, S], BF16, kind="ExternalInput")
    vT = nc.dram_tensor("vT", [D, S], BF16, kind="ExternalInput")
    wqhi = nc.dram_tensor("wqhi", [D, NH * DK], BF16, kind="ExternalInput")
    wqlo = nc.dram_tensor("wqlo", [D, NH * DK], BF16, kind="ExternalInput")
    wkhi = nc.dram_tensor("wkhi", [D, NH * DK], BF16, kind="ExternalInput")
    wklo = nc.dram_tensor("wklo", [D, NH * DK], BF16, kind="ExternalInput")
    wv = nc.dram_tensor("wv", [D, NH * DK], BF16, kind="ExternalInput")
    wo = nc.dram_tensor("wo", [NH * DK, D], DT_P, kind="ExternalInput")
    maskf = nc.dram_tensor("maskf", [P, P], BF16, kind="ExternalInput")
    maskb = nc.dram_tensor("maskb", [P, P], BF16, kind="ExternalInput")
    onesr = nc.dram_tensor("onesr", [1, S], F32R, kind="ExternalInput")
    y0 = nc.dram_tensor("y0", [S, D], BF16, kind="ExternalOutput")

    with tile.TileContext(nc) as tc:
        import contextlib

        ctx = contextlib.ExitStack()
        with ctx:
            const = ctx.enter_context(tc.tile_pool(name="const", bufs=1))
            big = ctx.enter_context(tc.tile_pool(name="big", bufs=1))
            stream = ctx.enter_context(
                tc.tile_pool(name="stream", bufs=int(os.environ.get("ATTN_BSTREAM", "6")))
            )
            pstream = ctx.enter_context(
                tc.tile_pool(name="pstream", bufs=int(os.environ.get("ATTN_BPS", "16")))
            )
            ptp = ctx.enter_context(
                tc.tile_pool(name="ptp", bufs=int(os.environ.get("ATTN_BPT", "3")))
            )
            ypool = ctx.enter_context(
                tc.tile_pool(name="ypool", bufs=int(os.environ.get("ATTN_BY", "2")))
            )
            smalls = ctx.enter_context(
                tc.tile_pool(name="smalls", bufs=int(os.environ.get("ATTN_BSM", "4")))
            )
            bproj = int(os.environ.get("ATTN_BPROJ", "2"))
            bstat = int(os.environ.get("ATTN_BSTAT", "2"))
            bst = int(os.environ.get("ATTN_BST", "2"))
            bmisc = int(os.environ.get("ATTN_BMISC", "2"))
            ps_proj = ctx.enter_context(
                tc.tile_pool(name="ps_proj", bufs=bproj, space="PSUM")
            )
            ps_stat = ctx.enter_context(
                tc.tile_pool(name="ps_stat", bufs=bstat, space="PSUM")
            )
            ps_st = ctx.enter_context(tc.tile_pool(name="ps_st", bufs=bst, space="PSUM"))
            ps_misc = ctx.enter_context(
                tc.tile_pool(name="ps_misc", bufs=bmisc, space="PSUM")
            )

            # ---- constants ----
            ident = const.tile([P, P], F32)
            make_identity(nc, ident[:])
            ident_b = const.tile([P, P], BF16)
            nc.vector.tensor_copy(ident_b[:], ident[:])
            ident_r = const.tile([P, P], F32R)
            nc.vector.tensor_copy(ident_r[:], ident[:])

            wq_hi_sb = const.tile([P, DC, P], BF16, tag="wqh")
            wq_lo_sb = const.tile([P, DC, P], BF16, tag="wql")
            wk_hi_sb = const.tile([P, DC, P], BF16, tag="wkh")
            wk_lo_sb = const.tile([P, DC, P], BF16, tag="wkl")
            wv_sb = const.tile([P, DC, P], BF16, tag="wv")
            nc.sync.dma_start(wq_hi_sb[:], wqhi.rearrange("(o p) m -> p o m", p=P))
            nc.sync.dma_start(wq_lo_sb[:], wqlo.rearrange("(o p) m -> p o m", p=P))
            nc.sync.dma_start(wk_hi_sb[:], wkhi.rearrange("(o p) m -> p o m", p=P))
            nc.sync.dma_start(wk_lo_sb[:], wklo.rearrange("(o p) m -> p o m", p=P))
            nc.sync.dma_start(wv_sb[:], wv.rearrange("(o p) m -> p o m", p=P))
            wo_sb = const.tile([P, D], DT_P, tag="wo")
            nc.sync.dma_start(wo_sb[:], wo[:])

            mf_sb = const.tile([P, P], BF16, tag="mf")
            mb_sb = const.tile([P, P], BF16, tag="mb")
            nc.sync.dma_start(mf_sb[:], maskf[:])
            nc.sync.dma_start(mb_sb[:], maskb[:])

            # ---- persistent activations ----
            # qTa[h]: rows 0-63 = Qh^T (f32r), row 64 = -m (written per qb by
            # stats), row 96 = l stash. kTa[h]: rows 0-63 = Kh^T, row 64 = ones.
            # qhT2/khT2: 2-head packed (h0 -> 0-63, h1 -> 64-127) for the
            # row-tiled stats matmuls.
            qTa = [big.tile([P, S], F32R, tag=f"qTa{h}", name=f"qTa{h}") for h in range(NH)]
            kTa = [big.tile([P, S], F32R, tag=f"kTa{h}", name=f"kTa{h}") for h in range(NH)]
            qhT2 = big.tile([P, S], DT_P, tag="qhT2", name="qhT2")
            khT2 = big.tile([P, S], DT_P, tag="khT2", name="khT2")
            vh = [big.tile([P, QB, DK + 1], DT_P, tag=f"vh{h}", name=f"vh{h}") for h in range(NH)]
            ct = big.tile([P, S], DT_P, tag="ct")
            mcol = [big.tile([P, QB], F32R, tag=f"mcol{h}", name=f"mcol{h}") for h in range(NH)]
            lcol = [big.tile([P, QB], F32R, tag=f"lcol{h}", name=f"lcol{h}") for h in range(NH)]
            rcol = [big.tile([P, QB], F32, tag=f"rcol{h}", name=f"rcol{h}") for h in range(NH)]

            ones_qb = const.tile([P, QB], F32, tag="ones_qb")
            nc.any.memset(ones_qb[:], 1.0)
            for h in range(NH):
                nc.sync.dma_start(kTa[h][DK : DK + 1, :], onesr[:])  # ones row
                nc.vector.tensor_copy(vh[h][:, :, DK], ones_qb[:])  # ones col

            for _rep in range(repeat):
                # ---- Q/K projection block emitters ----
                # fp32 accuracy via bf16 hi/lo pairs (host-split): X@W =
                # Xhi@Whi + Xhi@Wlo + Xlo@Whi (lo*lo term negligible), each
                # a 1-cycle/row bf16 matmul vs fp32's 4 cycles/row.
                def proj_emit(t_idx, nb):
                    xhl, w_hi, w_lo = [
                        (qThl, wq_hi_sb, wq_lo_sb),
                        (kThl, wk_hi_sb, wk_lo_sb),
                    ][t_idx]
                    ps = ps_proj.tile([P, 512], F32, tag="proj", name="ps")
                    for dc in range(DC):
                        xt = stream.tile([P, 2, 512], BF16, tag="xin", name="xt")
                        dsl = slice(dc * P, (dc + 1) * P)
                        nsl = slice(nb * 512, (nb + 1) * 512)
                        nc.sync.dma_start(xt[:], xhl[dsl, :, nsl])
                        nc.tensor.matmul(
                            ps[:], w_hi[:, dc, :], xt[:, 0, :],
                            start=(dc == 0), stop=False,
                        )
                        nc.tensor.matmul(
                            ps[:], w_lo[:, dc, :], xt[:, 0, :],
                            start=False, stop=False,
                        )
                        nc.tensor.matmul(
                            ps[:], w_hi[:, dc, :], xt[:, 1, :],
                            start=False, stop=(dc == DC - 1),
                        )
                    sl = slice(nb * 512, (nb + 1) * 512)
                    pk2 = qhT2 if t_idx == 0 else khT2
                    dst = qTa if t_idx == 0 else kTa
                    nc.scalar.copy(pk2[:, sl], ps[:])
                    for h in range(NH):
                        nc.scalar.copy(
                            dst[h][0:DK, sl], ps[h * DK : (h + 1) * DK, :]
                        )

                def vproj_emit(nb):
                    ps = ps_proj.tile([P, 512], F32, tag="proj", name="ps")
                    for dc in range(DC):
                        xt = stream.tile([P, 512], BF16, tag="xin", name="xtv")
                        nc.sync.dma_start(
                            xt[:],
                            vT[dc * P : (dc + 1) * P, nb * 512 : (nb + 1) * 512],
                        )
                        nc.tensor.matmul(
                            ps[:],
                            wv_sb[:, dc, :],
                            xt[:],
                            start=(dc == 0),
                            stop=(dc == DC - 1),
                        )
                    vtmp = stream.tile([P, 512], F32R, tag="xin", name="vtmp")
                    nc.vector.tensor_copy(vtmp[:], ps[:])
                    for h in range(NH):
                        pst = ps_misc.tile([P, 512], F32R, tag="misc", name="pst")
                        for j in range(4):
                            nc.tensor.transpose(
                                pst[0:P, j * DK : (j + 1) * DK],
                                vtmp[h * DK : (h + 1) * DK, j * P : (j + 1) * P],
                                ident_r[h * DK : (h + 1) * DK, h * DK : (h + 1) * DK],
                            )
                        nc.vector.tensor_copy(
                            vh[h][:, nb * 4 : nb * 4 + 4, 0:DK],
                            pst[:, 0 : 4 * DK].rearrange("p (j d) -> p j d", j=4),
                        )

                # ---- stats: row max via row-tiled 2-head matmuls ----
                # Emitted as fine-grained filler units popped between S^T kc
                # steps so the DVE reduces drain under S^T compute and the
                # two ps_stat banks recycle without stalling PE.
                ID = mybir.ActivationFunctionType.Identity
                from collections import deque

                stats_q = deque()  # (batch_nb, emit)
                wo_q = deque()
                proj_q = deque()  # (block_nb, emit) dc-chunk fillers
                mparts_of = {}
                acc_of = {}

                def proj_unit(t_idx, nb, dc, xt):
                    def emit():
                        key = (t_idx, nb)
                        w_hi, w_lo = [
                            (wq_hi_sb, wq_lo_sb),
                            (wk_hi_sb, wk_lo_sb),
                        ][t_idx]
                        if dc == 0:
                            acc_of[key] = ps_proj.tile(
                                [P, 512], F32, tag="proj", name="ps"
                            )
                        ps = acc_of[key]
                        nc.tensor.matmul(
                            ps[:], w_hi[:, dc, :], xt[:, 0, :],
                            start=(dc == 0), stop=False,
                        )
                        nc.tensor.matmul(
                            ps[:], w_lo[:, dc, :], xt[:, 0, :],
                            start=False, stop=False,
                        )
                        nc.tensor.matmul(
                            ps[:], w_hi[:, dc, :], xt[:, 1, :],
                            start=False, stop=(dc == DC - 1),
                        )
                        if dc == DC - 1:
                            del acc_of[key]
                            sl = slice(nb * 512, (nb + 1) * 512)
                            pk2 = qhT2 if t_idx == 0 else khT2
                            dst = qTa if t_idx == 0 else kTa
                            nc.scalar.copy(pk2[:, sl], ps[:])
                            for h in range(NH):
                                nc.scalar.copy(
                                    dst[h][0:DK, sl],
                                    ps[h * DK : (h + 1) * DK, :],
                                )

                    return emit

                def queue_proj(nb):
                    # DMAs are issued at queue time (one iteration ahead of
                    # consumption) so their latency hides under the S^T steps
                    for t_idx in (1, 0):  # K then Q
                        xhl = [qThl, kThl][t_idx]
                        for dc in range(DC):
                            xt = pstream.tile([P, 2, 512], BF16, tag="pxin", name="xt")
                            nc.sync.dma_start(
                                xt[:],
                                xhl[dc * P : (dc + 1) * P, :, nb * 512 : (nb + 1) * 512],
                            )
                            proj_q.append((nb, proj_unit(t_idx, nb, dc, xt)))

                def drain_proj(max_nb):
                    while proj_q and proj_q[0][0] <= max_nb:
                        proj_q.popleft()[1]()

                def pop_filler():
                    # proj before stats: stats units read the proj copies, so
                    # their matmuls must trail them in the PE stream
                    if proj_q:
                        proj_q.popleft()[1]()
                        return True
                    if stats_q:
                        stats_q.popleft()[1]()
                        return True
                    if wo_q:
                        wo_q.popleft()()
                        return True
                    return False

                def stats_unit(qb, kc):
                    def emit():
                        kmax = qb // 4 + 1 if causal else NB
                        if kc == 0:
                            mparts_of[qb] = [
                                smalls.tile([P, NB], F32, tag="mpart", name=f"mpart{h}")
                                for h in range(NH)
                            ]
                        mparts = mparts_of[qb]
                        diag = causal and (kc == qb // 4)
                        nv = (qb % 4) * P + P if diag else 512
                        pss = []
                        for h in range(NH):
                            ps = ps_stat.tile([P, 512], F32, tag="stat", name="ps_stat")
                            nc.tensor.matmul(
                                ps[:, 0:nv],
                                qhT2[h * DK : (h + 1) * DK, qb * P : (qb + 1) * P],
                                khT2[h * DK : (h + 1) * DK, kc * 512 : kc * 512 + nv],
                                start=True,
                                stop=not diag,
                            )
                            pss.append(ps)
                        for h in range(NH):
                            if diag:
                                nc.tensor.matmul(
                                    pss[h][:, nv - P : nv],
                                    ident_b[:],
                                    mb_sb[:],
                                    start=False,
                                    stop=True,
                                )
                            nc.vector.reduce_max(
                                mparts[h][:, kc : kc + 1],
                                pss[h][:, 0:nv],
                                axis=mybir.AxisListType.X,
                            )
                        if kc == kmax - 1:
                            del mparts_of[qb]
                            for h in range(NH):
                                nc.vector.tensor_reduce(
                                    mcol[h][:, qb : qb + 1],
                                    mparts[h][:, 0:kmax],
                                    axis=mybir.AxisListType.X,
                                    op=mybir.AluOpType.max,
                                    negate=True,
                                )
                                nc.sync.dma_start(
                                    qTa[h][DK : DK + 1, qb * P : (qb + 1) * P],
                                    mcol[h][:, qb : qb + 1],
                                )

                    return emit

                def queue_stats(batch):
                    for qb in range(4 * batch, 4 * batch + 4):
                        kmax = qb // 4 + 1 if causal else NB
                        for kc in range(kmax):
                            stats_q.append((batch, stats_unit(qb, kc)))

                def drain_stats(max_batch):
                    # force-complete stats units for batches <= max_batch
                    # (gates the S^T pass that reads their m rows)
                    while stats_q and stats_q[0][0] <= max_batch:
                        stats_q.popleft()[1]()

                # ---- S^T + exp + AV ----
                def st3_emit(h, nb):
                    nkc = 4 * (nb + 1) if causal else QB
                    po = ps_misc.tile([P, 512], F32, tag="misc", name="po")
                    pss = {}

                    def s_mm(kc):
                        ps = ps_st.tile([P, 512], F32, tag="st", name="ps_st")
                        diag = causal and (kc >= 4 * nb)
                        o = kc - 4 * nb if diag else 0
                        qoff = o * P
                        nv = 512 - qoff
                        kslice = slice(kc * P, (kc + 1) * P)
                        qslice = slice(nb * 512 + qoff, (nb + 1) * 512)
                        nc.tensor.matmul(
                            ps[:, 0:nv],
                            kTa[h][0 : DK + 1, kslice],
                            qTa[h][0 : DK + 1, qslice],
                            start=True,
                            stop=not diag,
                        )
                        if diag:
                            nc.tensor.matmul(
                                ps[:, 0:P],
                                ident_b[:],
                                mf_sb[:],
                                start=False,
                                stop=True,
                            )
                        pss[kc] = (ps, qoff, nv)

                    s_mm(0)
                    for kc in range(nkc):
                        if kc + 1 < nkc:
                            s_mm(kc + 1)
                        ps, qoff, nv = pss.pop(kc)
                        pt = ptp.tile([P, 512], DT_P, tag="pt", name="pt")
                        nc.scalar.activation(pt[:, 0:nv], ps[:, 0:nv], EXP)
                        nc.tensor.matmul(
                            po[0 : DK + 1, qoff:512],
                            vh[h][:, kc, :],
                            pt[:, 0:nv],
                            start=(kc == 0),
                            stop=(kc == nkc - 1),
                        )
                        pop_filler()
                        if len(proj_q) > 8:
                            pop_filler()
                    nc.scalar.copy(
                        ct[h * DK : (h + 1) * DK, nb * 512 : (nb + 1) * 512],
                        po[0:DK, :],
                    )
                    nc.vector.tensor_copy(
                        qTa[h][96:97, nb * 512 : (nb + 1) * 512],
                        po[DK : DK + 1, :],
                    )
                    for j in range(4):
                        qb = nb * 4 + j
                        nc.sync.dma_start(
                            lcol[h][:, qb : qb + 1],
                            qTa[h][96:97, qb * P : (qb + 1) * P],
                        )
                    nc.vector.reciprocal(
                        rcol[h][:, nb * 4 : nb * 4 + 4],
                        lcol[h][:, nb * 4 : nb * 4 + 4],
                    )

                def wo_unit(qc, eb):
                    def emit():
                        ysb0 = ypool.tile([P, 512], F32, tag="ysb0", name="ysb0")
                        ysb1 = ypool.tile([P, 512], F32, tag="ysb1", name="ysb1")
                        psys = []
                        for h in range(NH):
                            psy = ps_proj.tile([P, 512], F32, tag="proj", name="psy")
                            nc.tensor.matmul(
                                psy[:],
                                ct[h * DK : (h + 1) * DK, qc * P : (qc + 1) * P],
                                wo_sb[h * DK : (h + 1) * DK, eb * 512 : (eb + 1) * 512],
                                start=True,
                                stop=True,
                            )
                            psys.append(psy)
                        for h, ysb in ((0, ysb0), (1, ysb1)):
                            nc.scalar.activation(
                                ysb[:], psys[h][:], ID, scale=rcol[h][:, qc : qc + 1]
                            )
                        ysbo = ypool.tile([P, 512], BF16, tag="ysbo", name="ysbo")
                        nc.gpsimd.tensor_add(out=ysbo[:], in0=ysb0[:], in1=ysb1[:])
                        nc.sync.dma_start(
                            y0[qc * P : (qc + 1) * P, eb * 512 : (eb + 1) * 512],
                            ysbo[:],
                        )

                    return emit

                def queue_wo(qcs):
                    for qc in qcs:
                        for eb in range(2):
                            wo_q.append(wo_unit(qc, eb))

                # Fully fused (default): block 0 projected directly, every
                # later K/Q projection block enters as dc-chunk fillers popped
                # between S^T kc steps (priority proj > stats > wo), so PE
                # gets proj work while ACT paces exp and DVE drains stats
                # reduces. ATTN_FUSEPROJ=0 projects everything upfront.
                fuse_proj = bool(int(os.environ.get("ATTN_FUSEPROJ", "0")))
                if fuse_proj:
                    proj_emit(1, 0)  # K block 0
                    proj_emit(0, 0)  # Q block 0
                else:
                    for nb in range(NB):
                        proj_emit(1, nb)
                    for nb in range(NB):
                        proj_emit(0, nb)
                queue_stats(0)
                drain_stats(0)
                for nb in range(NB):
                    if nb + 1 < NB:
                        if fuse_proj:
                            queue_proj(nb + 1)
                        queue_stats(nb + 1)
                    if nb >= 1:
                        queue_wo(range((nb - 1) * 4, nb * 4))
                    vproj_emit(nb)
                    drain_proj(nb)
                    drain_stats(nb)
                    st3_emit(0, nb)
                    st3_emit(1, nb)
                queue_wo(range((NB - 1) * 4, NB * 4))
                while proj_q or stats_q or wo_q:
                    pop_filler()

    _split_waits(nc)
    return nc


_cache = {}


def _get_nc(causal: bool):
    if causal not in _cache:
        nc = bass.Bass(trn_type="TRN2")
        build(nc, causal=causal)
        _cache[causal] = nc
    return _cache[causal]


def _host_masks():
    p = np.arange(P)[:, None]
    j = np.arange(P)[None, :]
    # S^T diag tile [kc, q]: nonzero only in the first 128 q-cols: p > j
    maskf = np.where(p > j, NEG, 0.0).astype(ml_dtypes.bfloat16)
    # stats diag tile [q, kc]: nonzero only in the last 128 kc-cols: j > p
    maskb = np.where(j > p, NEG, 0.0).astype(ml_dtypes.bfloat16)
    return maskf, maskb


def make_in_maps(np_inputs):
    Q = np.asarray(np_inputs["Q"], dtype=np.float32)
    K = np.asarray(np_inputs["K"], dtype=np.float32)
    V = np.asarray(np_inputs["V"], dtype=np.float32)
    W_Q = np.asarray(np_inputs["W_Q"], dtype=np.float32)
    W_K = np.asarray(np_inputs["W_K"], dtype=np.float32)
    W_V = np.asarray(np_inputs["W_V"], dtype=np.float32)
    W_O = np.asarray(np_inputs["W_O"], dtype=np.float32)

    def bf16_pair(x):
        hi = x.astype(ml_dtypes.bfloat16)
        lo = (x - hi.astype(np.float32)).astype(ml_dtypes.bfloat16)
        return np.ascontiguousarray(hi), np.ascontiguousarray(lo)

    qThl = np.ascontiguousarray(np.stack(bf16_pair(Q.T), axis=1))
    kThl = np.ascontiguousarray(np.stack(bf16_pair(K.T), axis=1))
    vTh = np.ascontiguousarray(V.T.astype(ml_dtypes.bfloat16))
    maskf, maskb = _host_masks()
    ones_row = np.ones((1, S), dtype=np.float32)

    scale = np.float32(1.0 / np.sqrt(DK))
    in_maps = []
    for c in range(NCORES):
        h0, h1 = 2 * c, 2 * c + 1
        wq2 = np.ascontiguousarray(
            np.concatenate([W_Q[h0] * scale, W_Q[h1] * scale], axis=1)
        ).astype(np.float32)
        wk2 = np.ascontiguousarray(
            np.concatenate([W_K[h0], W_K[h1]], axis=1)
        ).astype(np.float32)
        wqhi, wqlo = bf16_pair(wq2)
        wkhi, wklo = bf16_pair(wk2)
        wv2 = np.ascontiguousarray(
            np.concatenate([W_V[h0], W_V[h1]], axis=1).astype(ml_dtypes.bfloat16)
        )
        wo2 = np.ascontiguousarray(W_O[P * c : P * (c + 1), :])
        if bool(int(os.environ.get("ATTN_BF16P", "1"))):
            wo2 = wo2.astype(ml_dtypes.bfloat16)
        in_maps.append(
            {
                "qThl": qThl,
                "kThl": kThl,
                "vT": vTh,
                "wqhi": wqhi,
                "wqlo": wqlo,
                "wkhi": wkhi,
                "wklo": wklo,
                "wv": wv2,
                "wo": wo2,
                "maskf": maskf,
                "maskb": maskb,
                "onesr": ones_row,
            }
        )
    return in_maps


LAST_EXEC_NS = None


def kernel(Q, K, V, W_Q, W_K, W_V, W_O, mask):
    global LAST_EXEC_NS
    causal = bool(np.asarray(mask).item())
    nc = _get_nc(causal)
    in_maps = make_in_maps(
        dict(Q=Q, K=K, V=V, W_Q=W_Q, W_K=W_K, W_V=W_V, W_O=W_O)
    )

    trace = bool(int(os.environ.get("ATTN_TRACE", "0")))
    res = run_bass_kernel_spmd(
        nc, in_maps, core_ids=list(range(NCORES)), trace=trace
    )
    LAST_EXEC_NS = res.exec_time_ns

    out = np.zeros((S, D), dtype=np.float32)
    for c in range(NCORES):
        out += res.results[c]["y0"].astype(np.float32)
    return out


# revision 46
# speedup vs baseline: 1.2395x; 1.1336x over previous
"""Multi-head causal attention (SEQ=4096, D=1024, H=16, DK=64) on 8 TRN2
NeuronCores, tensor-parallel over heads (2 heads/core). Self-contained.

Per-core pipeline (v2):
  1. Projections: Qh^T/Kh^T/Vh^T = W.T @ X^T in fp32 (X^T pre-transposed on
     host, 1/sqrt(dk) folded into W_Q host-side). PSUM results copied to f32r
     tiles: per-head qT/kT (rows 0-63 + aux rows) and 2-head packed
     qhT2/khT2 (h0 -> partitions 0-63, h1 -> 64-127) for row-tiled stats.
  2. Stats pass: S = Qh^T.T @ Kh^T, single f32r matmul per tile, the two
     heads' matmuls row-tiled via tile_position (0,0)/(64,0) so they run
     concurrently in disjoint array row-groups. Causal mask via
     identity-matmul of a -1e9 tile. Row-max reduced on DVE, negated, and
     DMA-transposed into row 64 of the per-head qT tile.
  3. S^T pass: single f32r matmul S^T[kc,q] = [Kh;1].T @ [Qh;-m] (the max
     subtraction rides the contraction as the 65th row). f32r operand
     rounding gives score error ~5e-2 abs (on a +-3500 range), which maps to
     ~2e-3 output rel err (empirically calibrated) - far inside the 2e-2
     gate. Mask via identity matmul, ACT exp -> P^T. AV in f32r:
     ones-augmented Vh gives l = sum(exp) as row 64 of the PSUM accumulator.
  4. 1/l per qb (DVE reciprocal), Y_partial = C^T.T @ W_O_rows with the two
     heads' matmuls row-tiled, scaled by 1/l on ACT, summed on DVE, DMA out.
     Host sums the 8 per-core partials.
"""

import os
import sys

sys.path.insert(0, "/opt/trn_rl_repo")

import numpy as np
import ml_dtypes

import concourse.bass as bass
import concourse.mybir as mybir
import concourse.tile as tile
from concourse.bass_utils import run_bass_kernel_spmd
from concourse.masks import make_identity

P = 128
S = 4096
D = 1024
DK = 64
NH = 2  # heads per core
NCORES = 8
NEG = -1.0e9
F32 = mybir.dt.float32
F32R = mybir.dt.float32r
BF16 = mybir.dt.bfloat16
EXP = mybir.ActivationFunctionType.Exp

_ctr = [0]


def _split_waits(nc, max_waits=1):
    """walrus rejects >1 sem-wait per instruction; move extras onto
    preceding same-engine NOPs (engine streams are program-ordered)."""
    for f in nc.m.functions:
        for bb in f.blocks:
            insts = bb.instructions
            new = []
            changed = False
            for inst in insts:
                si = inst.sync_info
                if si is not None and si.on_wait and len(si.on_wait) > max_waits:
                    waits = list(si.on_wait)
                    extra, keep = waits[:-max_waits], waits[-max_waits:]
                    for i in range(0, len(extra), max_waits):
                        _ctr[0] += 1
                        new.append(
                            mybir.InstNoOp(
                                name=f"waitsplit-{_ctr[0]}",
                                engine=inst.engine,
                                ins=[],
                                outs=[],
                                sync_info=mybir.SyncInfo(
                                    on_wait=extra[i : i + max_waits], on_update=[]
                                ),
                            )
                        )
                    inst.sync_info = mybir.SyncInfo(
                        on_wait=keep, on_update=list(si.on_update)
                    )
                    changed = True
                new.append(inst)
            if changed:
                bb.instructions = new


def build(nc: bass.Bass, causal: bool = True):
    repeat = int(os.environ.get("ATTN_REPEAT", "1"))
    # bf16 for the max-stats operands, P/V/C/W_O: bf16 matmuls get a
    # standalone pipelined LDWEIGHTS (f32r must self-load serially on HW)
    bf16p = bool(int(os.environ.get("ATTN_BF16P", "1")))
    DT_P = BF16 if bf16p else F32R
    NB = S // 512  # 8   512-wide blocks
    QB = S // P  # 32  128-wide q blocks
    DC = D // P  # 8   128-deep contraction chunks

    qThl = nc.dram_tensor("qThl", [D, 2, S], BF16, kind="ExternalInput")
    kThl = nc.dram_tensor("kThl", [D, 2, S], BF16, kind="ExternalInput")
    vT = nc.dram_tensor("vT", [D, S], BF16, kind="ExternalInput")
    wqhi = nc.dram_tensor("wqhi", [D, NH * DK], BF16, kind="ExternalInput")
    wqlo = nc.dram_tensor("wqlo", [D, NH * DK], BF16, kind="ExternalInput")
    wkhi = nc.dram_tensor("wkhi", [D, NH * DK], BF16, kind="ExternalInput")
    wklo = nc.dram_tensor("wklo", [D, NH * DK], BF16, kind="ExternalInput")
    wv = nc.dram_tensor("wv", [D, NH * DK], BF16, kind="ExternalInput")
    wo = nc.dram_tensor("wo", [NH * DK, D], DT_P, kind="ExternalInput")
    maskf = nc.dram_tensor("maskf", [P, P], BF16, kind="ExternalInput")
    maskb = nc.dram_tensor("maskb", [P, P], BF16, kind="ExternalInput")
    onesr = nc.dram_tensor("onesr", [1, S], F32R, kind="ExternalInput")
    y0 = nc.dram_tensor("y0", [S, D], BF16, kind="ExternalOutput")

    with tile.TileContext(nc) as tc:
        import contextlib

        ctx = contextlib.ExitStack()
        with ctx:
            const = ctx.enter_context(tc.tile_pool(name="const", bufs=1))
            big = ctx.enter_context(tc.tile_pool(name="big", bufs=1))
            stream = ctx.enter_context(
                tc.tile_pool(name="stream", bufs=int(os.environ.get("ATTN_BSTREAM", "6")))
            )
            pstream = ctx.enter_context(
                tc.tile_pool(name="pstream", bufs=int(os.environ.get("ATTN_BPS", "16")))
            )
            ptp = ctx.enter_context(
                tc.tile_pool(name="ptp", bufs=int(os.environ.get("ATTN_BPT", "3")))
            )
            ypool = ctx.enter_context(
                tc.tile_pool(name="ypool", bufs=int(os.environ.get("ATTN_BY", "2")))
            )
            smalls = ctx.enter_context(
                tc.tile_pool(name="smalls", bufs=int(os.environ.get("ATTN_BSM", "4")))
            )
            bproj = int(os.environ.get("ATTN_BPROJ", "2"))
            bstat = int(os.environ.get("ATTN_BSTAT", "2"))
            bst = int(os.environ.get("ATTN_BST", "2"))
            bmisc = int(os.environ.get("ATTN_BMISC", "2"))
            ps_proj = ctx.enter_context(
                tc.tile_pool(name="ps_proj", bufs=bproj, space="PSUM")
            )
            ps_stat = ctx.enter_context(
                tc.tile_pool(name="ps_stat", bufs=bstat, space="PSUM")
            )
            ps_st = ctx.enter_context(tc.tile_pool(name="ps_st", bufs=bst, space="PSUM"))
            ps_misc = ctx.enter_context(
                tc.tile_pool(name="ps_misc", bufs=bmisc, space="PSUM")
            )

            # ---- constants ----
            ident = const.tile([P, P], F32)
            make_identity(nc, ident[:])
            ident_b = const.tile([P, P], BF16)
            nc.vector.tensor_copy(ident_b[:], ident[:])
            ident_r = const.tile([P, P], F32R)
            nc.vector.tensor_copy(ident_r[:], ident[:])

            wq_hi_sb = const.tile([P, DC, P], BF16, tag="wqh")
            wq_lo_sb = const.tile([P, DC, P], BF16, tag="wql")
            wk_hi_sb = const.tile([P, DC, P], BF16, tag="wkh")
            wk_lo_sb = const.tile([P, DC, P], BF16, tag="wkl")
            wv_sb = const.tile([P, DC, P], BF16, tag="wv")
            nc.sync.dma_start(wq_hi_sb[:], wqhi.rearrange("(o p) m -> p o m", p=P))
            nc.sync.dma_start(wq_lo_sb[:], wqlo.rearrange("(o p) m -> p o m", p=P))
            nc.sync.dma_start(wk_hi_sb[:], wkhi.rearrange("(o p) m -> p o m", p=P))
            nc.sync.dma_start(wk_lo_sb[:], wklo.rearrange("(o p) m -> p o m", p=P))
            nc.sync.dma_start(wv_sb[:], wv.rearrange("(o p) m -> p o m", p=P))
            wo_sb = const.tile([P, D], DT_P, tag="wo")
            nc.sync.dma_start(wo_sb[:], wo[:])

            mf_sb = const.tile([P, P], BF16, tag="mf")
            mb_sb = const.tile([P, P], BF16, tag="mb")
            nc.sync.dma_start(mf_sb[:], maskf[:])
            nc.sync.dma_start(mb_sb[:], maskb[:])

            # ---- persistent activations ----
            # qTa[h]: rows 0-63 = Qh^T (f32r), row 64 = -m (written per qb by
            # stats), row 96 = l stash. kTa[h]: rows 0-63 = Kh^T, row 64 = ones.
            # qhT2/khT2: 2-head packed (h0 -> 0-63, h1 -> 64-127) for the
            # row-tiled stats matmuls.
            qTa = [big.tile([P, S], F32R, tag=f"qTa{h}", name=f"qTa{h}") for h in range(NH)]
            kTa = [big.tile([P, S], F32R, tag=f"kTa{h}", name=f"kTa{h}") for h in range(NH)]
            qhT2 = big.tile([P, S], DT_P, tag="qhT2", name="qhT2")
            khT2 = big.tile([P, S], DT_P, tag="khT2", name="khT2")
            vh = [big.tile([P, QB, DK + 1], DT_P, tag=f"vh{h}", name=f"vh{h}") for h in range(NH)]
            ct = big.tile([P, S], DT_P, tag="ct")
            mcol = [big.tile([P, QB], F32R, tag=f"mcol{h}", name=f"mcol{h}") for h in range(NH)]
            lcol = [big.tile([P, QB], F32R, tag=f"lcol{h}", name=f"lcol{h}") for h in range(NH)]
            rcol = [big.tile([P, QB], F32, tag=f"rcol{h}", name=f"rcol{h}") for h in range(NH)]

            ones_qb = const.tile([P, QB], F32, tag="ones_qb")
            nc.any.memset(ones_qb[:], 1.0)
            for h in range(NH):
                nc.sync.dma_start(kTa[h][DK : DK + 1, :], onesr[:])  # ones row
                nc.vector.tensor_copy(vh[h][:, :, DK], ones_qb[:])  # ones col

            for _rep in range(repeat):
                # ---- Q/K projection block emitters ----
                # fp32 accuracy via bf16 hi/lo pairs (host-split): X@W =
                # Xhi@Whi + Xhi@Wlo + Xlo@Whi (lo*lo term negligible), each
                # a 1-cycle/row bf16 matmul vs fp32's 4 cycles/row.
                def proj_emit(t_idx, nb):
                    xhl, w_hi, w_lo = [
                        (qThl, wq_hi_sb, wq_lo_sb),
                        (kThl, wk_hi_sb, wk_lo_sb),
                    ][t_idx]
                    ps = ps_proj.tile([P, 512], F32, tag="proj", name="ps")
                    for dc in range(DC):
                        xt = stream.tile([P, 2, 512], BF16, tag="xin", name="xt")
                        dsl = slice(dc * P, (dc + 1) * P)
                        nsl = slice(nb * 512, (nb + 1) * 512)
                        nc.sync.dma_start(xt[:], xhl[dsl, :, nsl])
                        nc.tensor.matmul(
                            ps[:], w_hi[:, dc, :], xt[:, 0, :],
                            start=(dc == 0), stop=False,
                        )
                        nc.tensor.matmul(
                            ps[:], w_lo[:, dc, :], xt[:, 0, :],
                            start=False, stop=False,
                        )
                        nc.tensor.matmul(
                            ps[:], w_hi[:, dc, :], xt[:, 1, :],
                            start=False, stop=(dc == DC - 1),
                        )
                    sl = slice(nb * 512, (nb + 1) * 512)
                    pk2 = qhT2 if t_idx == 0 else khT2
                    dst = qTa if t_idx == 0 else kTa
                    nc.scalar.copy(pk2[:, sl], ps[:])
                    for h in range(NH):
                        nc.scalar.copy(
                            dst[h][0:DK, sl], ps[h * DK : (h + 1) * DK, :]
                        )

                def vproj_emit(nb):
                    ps = ps_proj.tile([P, 512], F32, tag="proj", name="ps")
                    for dc in range(DC):
                        xt = stream.tile([P, 512], BF16, tag="xin", name="xtv")
                        nc.sync.dma_start(
                            xt[:],
                            vT[dc * P : (dc + 1) * P, nb * 512 : (nb + 1) * 512],
                        )
                        nc.tensor.matmul(
                            ps[:],
                            wv_sb[:, dc, :],
                            xt[:],
                            start=(dc == 0),
                            stop=(dc == DC - 1),
                        )
                    vtmp = stream.tile([P, 512], F32R, tag="xin", name="vtmp")
                    nc.vector.tensor_copy(vtmp[:], ps[:])
                    for h in range(NH):
                        pst = ps_misc.tile([P, 512], F32R, tag="misc", name="pst")
                        for j in range(4):
                            nc.tensor.transpose(
                                pst[0:P, j * DK : (j + 1) * DK],
                                vtmp[h * DK : (h + 1) * DK, j * P : (j + 1) * P],
                                ident_r[h * DK : (h + 1) * DK, h * DK : (h + 1) * DK],
                            )
                        nc.vector.tensor_copy(
                            vh[h][:, nb * 4 : nb * 4 + 4, 0:DK],
                            pst[:, 0 : 4 * DK].rearrange("p (j d) -> p j d", j=4),
                        )

                # ---- stats: row max via row-tiled 2-head matmuls ----
                # Emitted as fine-grained filler units popped between S^T kc
                # steps so the DVE reduces drain under S^T compute and the
                # two ps_stat banks recycle without stalling PE.
                ID = mybir.ActivationFunctionType.Identity
                from collections import deque

                stats_q = deque()  # (batch_nb, emit)
                wo_q = deque()
                proj_q = deque()  # (block_nb, emit) dc-chunk fillers
                mparts_of = {}
                acc_of = {}

                def proj_unit(t_idx, nb, dc, xt):
                    def emit():
                        key = (t_idx, nb)
                        w_hi, w_lo = [
                            (wq_hi_sb, wq_lo_sb),
                            (wk_hi_sb, wk_lo_sb),
                        ][t_idx]
                        if dc == 0:
                            acc_of[key] = ps_proj.tile(
                                [P, 512], F32, tag="proj", name="ps"
                            )
                        ps = acc_of[key]
                        nc.tensor.matmul(
                            ps[:], w_hi[:, dc, :], xt[:, 0, :],
                            start=(dc == 0), stop=False,
                        )
                        nc.tensor.matmul(
                            ps[:], w_lo[:, dc, :], xt[:, 0, :],
                            start=False, stop=False,
                        )
                        nc.tensor.matmul(
                            ps[:], w_hi[:, dc, :], xt[:, 1, :],
                            start=False, stop=(dc == DC - 1),
                        )
                        if dc == DC - 1:
                            del acc_of[key]
                            sl = slice(nb * 512, (nb + 1) * 512)
                            pk2 = qhT2 if t_idx == 0 else khT2
                            dst = qTa if t_idx == 0 else kTa
                            nc.scalar.copy(pk2[:, sl], ps[:])
                            for h in range(NH):
                                nc.scalar.copy(
                                    dst[h][0:DK, sl],
                                    ps[h * DK : (h + 1) * DK, :],
                                )

                    return emit

                def queue_proj(nb):
                    # DMAs are issued at queue time (one iteration ahead of
                    # consumption) so their latency hides under the S^T steps
                    for t_idx in (1, 0):  # K then Q
                        xhl = [qThl, kThl][t_idx]
                        for dc in range(DC):
                            xt = pstream.tile([P, 2, 512], BF16, tag="pxin", name="xt")
                            nc.sync.dma_start(
                                xt[:],
                                xhl[dc * P : (dc + 1) * P, :, nb * 512 : (nb + 1) * 512],
                            )
                            proj_q.append((nb, proj_unit(t_idx, nb, dc, xt)))

                def drain_proj(max_nb):
                    while proj_q and proj_q[0][0] <= max_nb:
                        proj_q.popleft()[1]()

                def pop_filler():
                    # proj before stats: stats units read the proj copies, so
                    # their matmuls must trail them in the PE stream
                    if proj_q:
                        proj_q.popleft()[1]()
                        return True
                    if stats_q:
                        stats_q.popleft()[1]()
                        return True
                    if wo_q:
                        wo_q.popleft()()
                        return True
                    return False

                def stats_unit(qb, kc):
                    def emit():
                        kmax = qb // 4 + 1 if causal else NB
                        if kc == 0:
                            mparts_of[qb] = [
                                smalls.tile([P, NB], F32, tag="mpart", name=f"mpart{h}")
                                for h in range(NH)
                            ]
                        mparts = mparts_of[qb]
                        diag = causal and (kc == qb // 4)
                        nv = (qb % 4) * P + P if diag else 512
                        pss = []
                        for h in range(NH):
                            ps = ps_stat.tile([P, 512], F32, tag="stat", name="ps_stat")
                            nc.tensor.matmul(
                                ps[:, 0:nv],
                                qhT2[h * DK : (h + 1) * DK, qb * P : (qb + 1) * P],
                                khT2[h * DK : (h + 1) * DK, kc * 512 : kc * 512 + nv],
                                start=True,
                                stop=not diag,
                            )
                            pss.append(ps)
                        for h in range(NH):
                            if diag:
                                nc.tensor.matmul(
                                    pss[h][:, nv - P : nv],
                                    ident_b[:],
                                    mb_sb[:],
                                    start=False,
                                    stop=True,
                                )
                            nc.vector.reduce_max(
                                mparts[h][:, kc : kc + 1],
                                pss[h][:, 0:nv],
                                axis=mybir.AxisListType.X,
                            )
                        if kc == kmax - 1:
                            del mparts_of[qb]
                            for h in range(NH):
                                nc.vector.tensor_reduce(
                                    mcol[h][:, qb : qb + 1],
                                    mparts[h][:, 0:kmax],
                                    axis=mybir.AxisListType.X,
                                    op=mybir.AluOpType.max,
                                    negate=True,
                                )
                                nc.sync.dma_start(
                                    qTa[h][DK : DK + 1, qb * P : (qb + 1) * P],
                                    mcol[h][:, qb : qb + 1],
                                )

                    return emit

                def queue_stats(batch):
                    for qb in range(4 * batch, 4 * batch + 4):
                        kmax = qb // 4 + 1 if causal else NB
                        for kc in range(kmax):
                            stats_q.append((batch, stats_unit(qb, kc)))

                def drain_stats(max_batch):
                    # force-complete stats units for batches <= max_batch
                    # (gates the S^T pass that reads their m rows)
                    while stats_q and stats_q[0][0] <= max_batch:
                        stats_q.popleft()[1]()

                # ---- S^T + exp + AV ----
                def st3_emit(h, nb):
                    nkc = 4 * (nb + 1) if causal else QB
                    po = ps_misc.tile([P, 512], F32, tag="misc", name="po")
                    pss = {}

                    def s_mm(kc):
                        ps = ps_st.tile([P, 512], F32, tag="st", name="ps_st")
                        diag = causal and (kc >= 4 * nb)
                        o = kc - 4 * nb if diag else 0
                        qoff = o * P
                        nv = 512 - qoff
                        kslice = slice(kc * P, (kc + 1) * P)
                        qslice = slice(nb * 512 + qoff, (nb + 1) * 512)
                        nc.tensor.matmul(
                            ps[:, 0:nv],
                            kTa[h][0 : DK + 1, kslice],
                            qTa[h][0 : DK + 1, qslice],
                            start=True,
                            stop=not diag,
                        )
                        if diag:
                            nc.tensor.matmul(
                                ps[:, 0:P],
                                ident_b[:],
                                mf_sb[:],
                                start=False,
                                stop=True,
                            )
                        pss[kc] = (ps, qoff, nv)

                    s_mm(0)
                    for kc in range(nkc):
                        if kc + 1 < nkc:
                            s_mm(kc + 1)
                        ps, qoff, nv = pss.pop(kc)
                        pt = ptp.tile([P, 512], DT_P, tag="pt", name="pt")
                        nc.scalar.activation(pt[:, 0:nv], ps[:, 0:nv], EXP)
                        nc.tensor.matmul(
                            po[0 : DK + 1, qoff:512],
                            vh[h][:, kc, :],
                            pt[:, 0:nv],
                            start=(kc == 0),
                            stop=(kc == nkc - 1),
                        )
                        pop_filler()
                        if len(proj_q) > 8:
                            pop_filler()
                    nc.scalar.copy(
                        ct[h * DK : (h + 1) * DK, nb * 512 : (nb + 1) * 512],
                        po[0:DK, :],
                    )
                    nc.vector.tensor_copy(
                        qTa[h][96:97, nb * 512 : (nb + 1) * 512],
                        po[DK : DK + 1, :],
                    )
                    for j in range(4):
                        qb = nb * 4 + j
                        nc.sync.dma_start(
                            lcol[h][:, qb : qb + 1],
                            qTa[h][96:97, qb * P : (qb + 1) * P],
                        )
                    nc.vector.reciprocal(
                        rcol[h][:, nb * 4 : nb * 4 + 4],
                        lcol[h][:, nb * 4 : nb * 4 + 4],
                    )

                def wo_unit(qc, eb):
                    def emit():
                        ysb0 = ypool.tile([P, 512], F32, tag="ysb0", name="ysb0")
                        ysb1 = ypool.tile([P, 512], F32, tag="ysb1", name="ysb1")
                        psys = []
                        for h in range(NH):
                            psy = ps_proj.tile([P, 512], F32, tag="proj", name="psy")
                            nc.tensor.matmul(
                                psy[:],
                                ct[h * DK : (h + 1) * DK, qc * P : (qc + 1) * P],
                                wo_sb[h * DK : (h + 1) * DK, eb * 512 : (eb + 1) * 512],
                                start=True,
                                stop=True,
                            )
                            psys.append(psy)
                        for h, ysb in ((0, ysb0), (1, ysb1)):
                            nc.scalar.activation(
                                ysb[:], psys[h][:], ID, scale=rcol[h][:, qc : qc + 1]
                            )
                        ysbo = ypool.tile([P, 512], BF16, tag="ysbo", name="ysbo")
                        nc.gpsimd.tensor_add(out=ysbo[:], in0=ysb0[:], in1=ysb1[:])
                        nc.sync.dma_start(
                            y0[qc * P : (qc + 1) * P, eb * 512 : (eb + 1) * 512],
                            ysbo[:],
                        )

                    return emit

                def queue_wo(qcs):
                    for qc in qcs:
                        for eb in range(2):
                            wo_q.append(wo_unit(qc, eb))

                # Fully fused (default): block 0 projected directly, every
                # later K/Q projection block enters as dc-chunk fillers popped
                # between S^T kc steps (priority proj > stats > wo), so PE
                # gets proj work while ACT paces exp and DVE drains stats
                # reduces. ATTN_FUSEPROJ=0 projects everything upfront.
                fuse_proj = bool(int(os.environ.get("ATTN_FUSEPROJ", "0")))
                if fuse_proj:
                    proj_emit(1, 0)  # K block 0
                    proj_emit(0, 0)  # Q block 0
                else:
                    for nb in range(NB):
                        proj_emit(1, nb)
                    for nb in range(NB):
                        proj_emit(0, nb)
                queue_stats(0)
                drain_stats(0)
                for nb in range(NB):
                    if nb + 1 < NB:
                        if fuse_proj:
                            queue_proj(nb + 1)
                        queue_stats(nb + 1)
                    if nb >= 1:
                        queue_wo(range((nb - 1) * 4, nb * 4))
                    vproj_emit(nb)
                    drain_proj(nb)
                    drain_stats(nb)
                    st3_emit(0, nb)
                    st3_emit(1, nb)
                queue_wo(range((NB - 1) * 4, NB * 4))
                while proj_q or stats_q or wo_q:
                    pop_filler()

    _split_waits(nc)
    return nc


_cache = {}


def _get_nc(causal: bool):
    if causal not in _cache:
        nc = bass.Bass(trn_type="TRN2")
        build(nc, causal=causal)
        _cache[causal] = nc
    return _cache[causal]


def _host_masks():
    p = np.arange(P)[:, None]
    j = np.arange(P)[None, :]
    # S^T diag tile [kc, q]: nonzero only in the first 128 q-cols: p > j
    maskf = np.where(p > j, NEG, 0.0).astype(ml_dtypes.bfloat16)
    # stats diag tile [q, kc]: nonzero only in the last 128 kc-cols: j > p
    maskb = np.where(j > p, NEG, 0.0).astype(ml_dtypes.bfloat16)
    return maskf, maskb


def make_in_maps(np_inputs):
    Q = np.asarray(np_inputs["Q"], dtype=np.float32)
    K = np.asarray(np_inputs["K"], dtype=np.float32)
    V = np.asarray(np_inputs["V"], dtype=np.float32)
    W_Q = np.asarray(np_inputs["W_Q"], dtype=np.float32)
    W_K = np.asarray(np_inputs["W_K"], dtype=np.float32)
    W_V = np.asarray(np_inputs["W_V"], dtype=np.float32)
    W_O = np.asarray(np_inputs["W_O"], dtype=np.float32)

    def bf16_pair(x):
        hi = x.astype(ml_dtypes.bfloat16)
        lo = (x - hi.astype(np.float32)).astype(ml_dtypes.bfloat16)
        return np.ascontiguousarray(hi), np.ascontiguousarray(lo)

    qThl = np.ascontiguousarray(np.stack(bf16_pair(Q.T), axis=1))
    kThl = np.ascontiguousarray(np.stack(bf16_pair(K.T), axis=1))
    vTh = np.ascontiguousarray(V.T.astype(ml_dtypes.bfloat16))
    maskf, maskb = _host_masks()
    ones_row = np.ones((1, S), dtype=np.float32)

    scale = np.float32(1.0 / np.sqrt(DK))
    in_maps = []
    for c in range(NCORES):
        h0, h1 = 2 * c, 2 * c + 1
        wq2 = np.ascontiguousarray(
            np.concatenate([W_Q[h0] * scale, W_Q[h1] * scale], axis=1)
        ).astype(np.float32)
        wk2 = np.ascontiguousarray(
            np.concatenate([W_K[h0], W_K[h1]], axis=1)
        ).astype(np.float32)
        wqhi, wqlo = bf16_pair(wq2)
        wkhi, wklo = bf16_pair(wk2)
        wv2 = np.ascontiguousarray(
            np.concatenate([W_V[h0], W_V[h1]], axis=1).astype(ml_dtypes.bfloat16)
        )
        wo2 = np.ascontiguousarray(W_O[P * c : P * (c + 1), :])
        if bool(int(os.environ.get("ATTN_BF16P", "1"))):
            wo2 = wo2.astype(ml_dtypes.bfloat16)
        in_maps.append(
            {
                "qThl": qThl,
                "kThl": kThl,
                "vT": vTh,
                "wqhi": wqhi,
                "wqlo": wqlo,
                "wkhi": wkhi,
                "wklo": wklo,
                "wv": wv2,
                "wo": wo2,
                "maskf": maskf,
                "maskb": maskb,
                "onesr": ones_row,
            }
        )
    return in_maps


LAST_EXEC_NS = None


def kernel(Q, K, V, W_Q, W_K, W_V, W_O, mask):
    global LAST_EXEC_NS
    causal = bool(np.asarray(mask).item())
    nc = _get_nc(causal)
    in_maps = make_in_maps(
        dict(Q=Q, K=K, V=V, W_Q=W_Q, W_K=W_K, W_V=W_V, W_O=W_O)
    )

    trace = bool(int(os.environ.get("ATTN_TRACE", "0")))
    res = run_bass_kernel_spmd(
        nc, in_maps, core_ids=list(range(NCORES)), trace=trace
    )
    LAST_EXEC_NS = res.exec_time_ns

    out = np.zeros((S, D), dtype=np.float32)
    for c in range(NCORES):
        out += res.results[c]["y0"].astype(np.float32)
    return out


# revision 50
# speedup vs baseline: 1.4446x; 1.1654x over previous
"""Multi-head causal attention (SEQ=4096, D=1024, H=16, DK=64) on 8 TRN2
NeuronCores, tensor-parallel over heads (2 heads/core). Self-contained.

Per-core pipeline (v2):
  1. Projections: Qh^T/Kh^T/Vh^T = W.T @ X^T in fp32 (X^T pre-transposed on
     host, 1/sqrt(dk) folded into W_Q host-side). PSUM results copied to f32r
     tiles: per-head qT/kT (rows 0-63 + aux rows) and 2-head packed
     qhT2/khT2 (h0 -> partitions 0-63, h1 -> 64-127) for row-tiled stats.
  2. Stats pass: S = Qh^T.T @ Kh^T, single f32r matmul per tile, the two
     heads' matmuls row-tiled via tile_position (0,0)/(64,0) so they run
     concurrently in disjoint array row-groups. Causal mask via
     identity-matmul of a -1e9 tile. Row-max reduced on DVE, negated, and
     DMA-transposed into row 64 of the per-head qT tile.
  3. S^T pass: single f32r matmul S^T[kc,q] = [Kh;1].T @ [Qh;-m] (the max
     subtraction rides the contraction as the 65th row). f32r operand
     rounding gives score error ~5e-2 abs (on a +-3500 range), which maps to
     ~2e-3 output rel err (empirically calibrated) - far inside the 2e-2
     gate. Mask via identity matmul, ACT exp -> P^T. AV in f32r:
     ones-augmented Vh gives l = sum(exp) as row 64 of the PSUM accumulator.
  4. 1/l per qb (DVE reciprocal), Y_partial = C^T.T @ W_O_rows with the two
     heads' matmuls row-tiled, scaled by 1/l on ACT, summed on DVE, DMA out.
     Host sums the 8 per-core partials.
"""

import os
import sys

sys.path.insert(0, "/opt/trn_rl_repo")

import numpy as np
import ml_dtypes

import concourse.bass as bass
import concourse.mybir as mybir
import concourse.tile as tile
from concourse.bass_utils import run_bass_kernel_spmd
from concourse.masks import make_identity

P = 128
S = 4096
D = 1024
DK = 64
NH = 2  # heads per core
NCORES = 8
NEG = -1.0e9
F32 = mybir.dt.float32
F32R = mybir.dt.float32r
BF16 = mybir.dt.bfloat16
EXP = mybir.ActivationFunctionType.Exp

_ctr = [0]


def _split_waits(nc, max_waits=1):
    """walrus rejects >1 sem-wait per instruction; move extras onto
    preceding same-engine NOPs (engine streams are program-ordered)."""
    for f in nc.m.functions:
        for bb in f.blocks:
            insts = bb.instructions
            new = []
            changed = False
            for inst in insts:
                si = inst.sync_info
                if si is not None and si.on_wait and len(si.on_wait) > max_waits:
                    waits = list(si.on_wait)
                    extra, keep = waits[:-max_waits], waits[-max_waits:]
                    for i in range(0, len(extra), max_waits):
                        _ctr[0] += 1
                        new.append(
                            mybir.InstNoOp(
                                name=f"waitsplit-{_ctr[0]}",
                                engine=inst.engine,
                                ins=[],
                                outs=[],
                                sync_info=mybir.SyncInfo(
                                    on_wait=extra[i : i + max_waits], on_update=[]
                                ),
                            )
                        )
                    inst.sync_info = mybir.SyncInfo(
                        on_wait=keep, on_update=list(si.on_update)
                    )
                    changed = True
                new.append(inst)
            if changed:
                bb.instructions = new


def build(nc: bass.Bass, causal: bool = True):
    repeat = int(os.environ.get("ATTN_REPEAT", "1"))
    # bf16 for the max-stats operands, P/V/C/W_O: bf16 matmuls get a
    # standalone pipelined LDWEIGHTS (f32r must self-load serially on HW)
    bf16p = bool(int(os.environ.get("ATTN_BF16P", "1")))
    DT_P = BF16 if bf16p else F32R
    NB = S // 512  # 8   512-wide blocks
    QB = S // P  # 32  128-wide q blocks
    DC = D // P  # 8   128-deep contraction chunks

    qThl = nc.dram_tensor("qThl", [D, 2, S], BF16, kind="ExternalInput")
    kThl = nc.dram_tensor("kThl", [D, 2, S], BF16, kind="ExternalInput")
    vT = nc.dram_tensor("vT", [D, S], BF16, kind="ExternalInput")
    wqhi = nc.dram_tensor("wqhi", [D, NH * DK], BF16, kind="ExternalInput")
    wqlo = nc.dram_tensor("wqlo", [D, NH * DK], BF16, kind="ExternalInput")
    wkhi = nc.dram_tensor("wkhi", [D, NH * DK], BF16, kind="ExternalInput")
    wklo = nc.dram_tensor("wklo", [D, NH * DK], BF16, kind="ExternalInput")
    wv = nc.dram_tensor("wv", [D, NH * DK], BF16, kind="ExternalInput")
    wo = nc.dram_tensor("wo", [NH * DK, D], DT_P, kind="ExternalInput")
    maskf = nc.dram_tensor("maskf", [P, P], BF16, kind="ExternalInput")
    maskb = nc.dram_tensor("maskb", [P, P], BF16, kind="ExternalInput")
    onesr = nc.dram_tensor("onesr", [1, S], F32R, kind="ExternalInput")
    y0 = nc.dram_tensor("y0", [S, D], BF16, kind="ExternalOutput")

    with tile.TileContext(nc) as tc:
        import contextlib

        ctx = contextlib.ExitStack()
        with ctx:
            const = ctx.enter_context(tc.tile_pool(name="const", bufs=1))
            big = ctx.enter_context(tc.tile_pool(name="big", bufs=1))
            stream = ctx.enter_context(
                tc.tile_pool(name="stream", bufs=int(os.environ.get("ATTN_BSTREAM", "6")))
            )
            pstream = ctx.enter_context(
                tc.tile_pool(name="pstream", bufs=int(os.environ.get("ATTN_BPS", "16")))
            )
            ptp = ctx.enter_context(
                tc.tile_pool(name="ptp", bufs=int(os.environ.get("ATTN_BPT", "3")))
            )
            ypool = ctx.enter_context(
                tc.tile_pool(name="ypool", bufs=int(os.environ.get("ATTN_BY", "2")))
            )
            smalls = ctx.enter_context(
                tc.tile_pool(name="smalls", bufs=int(os.environ.get("ATTN_BSM", "4")))
            )
            bproj = int(os.environ.get("ATTN_BPROJ", "2"))
            bstat = int(os.environ.get("ATTN_BSTAT", "2"))
            bst = int(os.environ.get("ATTN_BST", "2"))
            bmisc = int(os.environ.get("ATTN_BMISC", "2"))
            ps_proj = ctx.enter_context(
                tc.tile_pool(name="ps_proj", bufs=bproj, space="PSUM")
            )
            ps_stat = ctx.enter_context(
                tc.tile_pool(name="ps_stat", bufs=bstat, space="PSUM")
            )
            ps_st = ctx.enter_context(tc.tile_pool(name="ps_st", bufs=bst, space="PSUM"))
            ps_misc = ctx.enter_context(
                tc.tile_pool(name="ps_misc", bufs=bmisc, space="PSUM")
            )

            # ---- constants ----
            ident = const.tile([P, P], F32)
            make_identity(nc, ident[:])
            ident_b = const.tile([P, P], BF16)
            nc.vector.tensor_copy(ident_b[:], ident[:])
            ident_r = const.tile([P, P], F32R)
            nc.vector.tensor_copy(ident_r[:], ident[:])

            wq_hi_sb = const.tile([P, DC, P], BF16, tag="wqh")
            wq_lo_sb = const.tile([P, DC, P], BF16, tag="wql")
            wk_hi_sb = const.tile([P, DC, P], BF16, tag="wkh")
            wk_lo_sb = const.tile([P, DC, P], BF16, tag="wkl")
            wv_sb = const.tile([P, DC, P], BF16, tag="wv")
            nc.sync.dma_start(wq_hi_sb[:], wqhi.rearrange("(o p) m -> p o m", p=P))
            nc.sync.dma_start(wq_lo_sb[:], wqlo.rearrange("(o p) m -> p o m", p=P))
            nc.sync.dma_start(wk_hi_sb[:], wkhi.rearrange("(o p) m -> p o m", p=P))
            nc.sync.dma_start(wk_lo_sb[:], wklo.rearrange("(o p) m -> p o m", p=P))
            nc.sync.dma_start(wv_sb[:], wv.rearrange("(o p) m -> p o m", p=P))
            wo_sb = const.tile([P, D], DT_P, tag="wo")
            nc.sync.dma_start(wo_sb[:], wo[:])

            mf_sb = const.tile([P, P], BF16, tag="mf")
            mb_sb = const.tile([P, P], BF16, tag="mb")
            nc.sync.dma_start(mf_sb[:], maskf[:])
            nc.sync.dma_start(mb_sb[:], maskb[:])

            # ---- persistent activations ----
            # qTa[h]: rows 0-63 = Qh^T (f32r), row 64 = -m (written per qb by
            # stats), row 96 = l stash. kTa[h]: rows 0-63 = Kh^T, row 64 = ones.
            # qhT2/khT2: 2-head packed (h0 -> 0-63, h1 -> 64-127) for the
            # row-tiled stats matmuls.
            qTa = [big.tile([P, S], F32R, tag=f"qTa{h}", name=f"qTa{h}") for h in range(NH)]
            kTa = [big.tile([P, S], F32R, tag=f"kTa{h}", name=f"kTa{h}") for h in range(NH)]
            qhT2 = big.tile([P, S], DT_P, tag="qhT2", name="qhT2")
            khT2 = big.tile([P, S], DT_P, tag="khT2", name="khT2")
            vh = [big.tile([P, QB, DK + 1], DT_P, tag=f"vh{h}", name=f"vh{h}") for h in range(NH)]
            ct = big.tile([P, S], DT_P, tag="ct")
            mcol = [big.tile([P, QB], F32R, tag=f"mcol{h}", name=f"mcol{h}") for h in range(NH)]
            lcol = [big.tile([P, QB], F32R, tag=f"lcol{h}", name=f"lcol{h}") for h in range(NH)]
            rcol = [big.tile([P, QB], F32, tag=f"rcol{h}", name=f"rcol{h}") for h in range(NH)]

            ones_qb = const.tile([P, QB], F32, tag="ones_qb")
            nc.any.memset(ones_qb[:], 1.0)
            for h in range(NH):
                nc.sync.dma_start(kTa[h][DK : DK + 1, :], onesr[:])  # ones row
                nc.vector.tensor_copy(vh[h][:, :, DK], ones_qb[:])  # ones col

            for _rep in range(repeat):
                # ---- Q/K projection block emitters ----
                # fp32 accuracy via bf16 hi/lo pairs (host-split): X@W =
                # Xhi@Whi + Xhi@Wlo + Xlo@Whi (lo*lo term negligible), each
                # a 1-cycle/row bf16 matmul vs fp32's 4 cycles/row.
                def proj_emit(t_idx, nb):
                    xhl, w_hi, w_lo = [
                        (qThl, wq_hi_sb, wq_lo_sb),
                        (kThl, wk_hi_sb, wk_lo_sb),
                    ][t_idx]
                    ps = ps_proj.tile([P, 512], F32, tag="proj", name="ps")
                    for dc in range(DC):
                        xt = stream.tile([P, 2, 512], BF16, tag="xin", name="xt")
                        dsl = slice(dc * P, (dc + 1) * P)
                        nsl = slice(nb * 512, (nb + 1) * 512)
                        nc.sync.dma_start(xt[:], xhl[dsl, :, nsl])
                        nc.tensor.matmul(
                            ps[:], w_hi[:, dc, :], xt[:, 0, :],
                            start=(dc == 0), stop=False,
                        )
                        nc.tensor.matmul(
                            ps[:], w_lo[:, dc, :], xt[:, 0, :],
                            start=False, stop=False,
                        )
                        nc.tensor.matmul(
                            ps[:], w_hi[:, dc, :], xt[:, 1, :],
                            start=False, stop=(dc == DC - 1),
                        )
                    sl = slice(nb * 512, (nb + 1) * 512)
                    pk2 = qhT2 if t_idx == 0 else khT2
                    dst = qTa if t_idx == 0 else kTa
                    nc.scalar.copy(pk2[:, sl], ps[:])
                    for h in range(NH):
                        nc.scalar.copy(
                            dst[h][0:DK, sl], ps[h * DK : (h + 1) * DK, :]
                        )

                def vproj_emit(nb):
                    ps = ps_proj.tile([P, 512], F32, tag="proj", name="ps")
                    for dc in range(DC):
                        xt = stream.tile([P, 512], BF16, tag="xin", name="xtv")
                        nc.sync.dma_start(
                            xt[:],
                            vT[dc * P : (dc + 1) * P, nb * 512 : (nb + 1) * 512],
                        )
                        nc.tensor.matmul(
                            ps[:],
                            wv_sb[:, dc, :],
                            xt[:],
                            start=(dc == 0),
                            stop=(dc == DC - 1),
                        )
                    vtmp = stream.tile([P, 512], F32R, tag="xin", name="vtmp")
                    nc.vector.tensor_copy(vtmp[:], ps[:])
                    for h in range(NH):
                        pst = ps_misc.tile([P, 512], F32R, tag="misc", name="pst")
                        for j in range(4):
                            nc.tensor.transpose(
                                pst[0:P, j * DK : (j + 1) * DK],
                                vtmp[h * DK : (h + 1) * DK, j * P : (j + 1) * P],
                                ident_r[h * DK : (h + 1) * DK, h * DK : (h + 1) * DK],
                            )
                        nc.vector.tensor_copy(
                            vh[h][:, nb * 4 : nb * 4 + 4, 0:DK],
                            pst[:, 0 : 4 * DK].rearrange("p (j d) -> p j d", j=4),
                        )

                # ---- stats: row max via row-tiled 2-head matmuls ----
                # Emitted as fine-grained filler units popped between S^T kc
                # steps so the DVE reduces drain under S^T compute and the
                # two ps_stat banks recycle without stalling PE.
                ID = mybir.ActivationFunctionType.Identity
                from collections import deque

                stats_q = deque()  # (batch_nb, emit)
                wo_q = deque()
                proj_q = deque()  # (block_nb, emit) dc-chunk fillers
                mparts_of = {}
                acc_of = {}

                def proj_unit(t_idx, nb, dc, xt):
                    def emit():
                        key = (t_idx, nb)
                        w_hi, w_lo = [
                            (wq_hi_sb, wq_lo_sb),
                            (wk_hi_sb, wk_lo_sb),
                        ][t_idx]
                        if dc == 0:
                            acc_of[key] = ps_proj.tile(
                                [P, 512], F32, tag="proj", name="ps"
                            )
                        ps = acc_of[key]
                        nc.tensor.matmul(
                            ps[:], w_hi[:, dc, :], xt[:, 0, :],
                            start=(dc == 0), stop=False,
                        )
                        nc.tensor.matmul(
                            ps[:], w_lo[:, dc, :], xt[:, 0, :],
                            start=False, stop=False,
                        )
                        nc.tensor.matmul(
                            ps[:], w_hi[:, dc, :], xt[:, 1, :],
                            start=False, stop=(dc == DC - 1),
                        )
                        if dc == DC - 1:
                            del acc_of[key]
                            sl = slice(nb * 512, (nb + 1) * 512)
                            pk2 = qhT2 if t_idx == 0 else khT2
                            dst = qTa if t_idx == 0 else kTa
                            nc.scalar.copy(pk2[:, sl], ps[:])
                            for h in range(NH):
                                nc.scalar.copy(
                                    dst[h][0:DK, sl],
                                    ps[h * DK : (h + 1) * DK, :],
                                )

                    return emit

                def queue_proj(nb):
                    # DMAs are issued at queue time (one iteration ahead of
                    # consumption) so their latency hides under the S^T steps
                    for t_idx in (1, 0):  # K then Q
                        xhl = [qThl, kThl][t_idx]
                        for dc in range(DC):
                            xt = pstream.tile([P, 2, 512], BF16, tag="pxin", name="xt")
                            nc.sync.dma_start(
                                xt[:],
                                xhl[dc * P : (dc + 1) * P, :, nb * 512 : (nb + 1) * 512],
                            )
                            proj_q.append((nb, proj_unit(t_idx, nb, dc, xt)))

                def drain_proj(max_nb):
                    while proj_q and proj_q[0][0] <= max_nb:
                        proj_q.popleft()[1]()

                def pop_filler():
                    # proj before stats: stats units read the proj copies, so
                    # their matmuls must trail them in the PE stream
                    if proj_q:
                        proj_q.popleft()[1]()
                        return True
                    if stats_q:
                        stats_q.popleft()[1]()
                        return True
                    if wo_q:
                        wo_q.popleft()()
                        return True
                    return False

                def stats_unit(qb, kc):
                    def emit():
                        kmax = qb // 4 + 1 if causal else NB
                        if kc == 0:
                            mparts_of[qb] = [
                                smalls.tile([P, NB], F32, tag="mpart", name=f"mpart{h}")
                                for h in range(NH)
                            ]
                        mparts = mparts_of[qb]
                        diag = causal and (kc == qb // 4)
                        nv = (qb % 4) * P + P if diag else 512
                        pss = []
                        for h in range(NH):
                            ps = ps_stat.tile([P, 512], F32, tag="stat", name="ps_stat")
                            nc.tensor.matmul(
                                ps[:, 0:nv],
                                qhT2[h * DK : (h + 1) * DK, qb * P : (qb + 1) * P],
                                khT2[h * DK : (h + 1) * DK, kc * 512 : kc * 512 + nv],
                                start=True,
                                stop=not diag,
                            )
                            pss.append(ps)
                        for h in range(NH):
                            if diag:
                                nc.tensor.matmul(
                                    pss[h][:, nv - P : nv],
                                    ident_b[:],
                                    mb_sb[:],
                                    start=False,
                                    stop=True,
                                )
                            nc.vector.reduce_max(
                                mparts[h][:, kc : kc + 1],
                                pss[h][:, 0:nv],
                                axis=mybir.AxisListType.X,
                            )
                        if kc == kmax - 1:
                            del mparts_of[qb]
                            for h in range(NH):
                                nc.vector.tensor_reduce(
                                    mcol[h][:, qb : qb + 1],
                                    mparts[h][:, 0:kmax],
                                    axis=mybir.AxisListType.X,
                                    op=mybir.AluOpType.max,
                                    negate=True,
                                )
                                nc.sync.dma_start(
                                    qTa[h][DK : DK + 1, qb * P : (qb + 1) * P],
                                    mcol[h][:, qb : qb + 1],
                                )

                    return emit

                def queue_stats(batch):
                    for qb in range(4 * batch, 4 * batch + 4):
                        kmax = qb // 4 + 1 if causal else NB
                        for kc in range(kmax):
                            stats_q.append((batch, stats_unit(qb, kc)))

                def drain_stats(max_batch):
                    # force-complete stats units for batches <= max_batch
                    # (gates the S^T pass that reads their m rows)
                    while stats_q and stats_q[0][0] <= max_batch:
                        stats_q.popleft()[1]()

                # ---- S^T + exp + AV ----
                def st3_emit(h, nb):
                    nkc = 4 * (nb + 1) if causal else QB
                    po = ps_misc.tile([P, 512], F32, tag="misc", name="po")
                    pss = {}

                    def s_mm(kc):
                        ps = ps_st.tile([P, 512], F32, tag="st", name="ps_st")
                        diag = causal and (kc >= 4 * nb)
                        o = kc - 4 * nb if diag else 0
                        qoff = o * P
                        nv = 512 - qoff
                        kslice = slice(kc * P, (kc + 1) * P)
                        qslice = slice(nb * 512 + qoff, (nb + 1) * 512)
                        nc.tensor.matmul(
                            ps[:, 0:nv],
                            kTa[h][0 : DK + 1, kslice],
                            qTa[h][0 : DK + 1, qslice],
                            start=True,
                            stop=not diag,
                        )
                        if diag:
                            nc.tensor.matmul(
                                ps[:, 0:P],
                                ident_b[:],
                                mf_sb[:],
                                start=False,
                                stop=True,
                            )
                        pss[kc] = (ps, qoff, nv)

                    s_mm(0)
                    for kc in range(nkc):
                        if kc + 1 < nkc:
                            s_mm(kc + 1)
                        ps, qoff, nv = pss.pop(kc)
                        pt = ptp.tile([P, 512], DT_P, tag="pt", name="pt")
                        nc.scalar.activation(pt[:, 0:nv], ps[:, 0:nv], EXP)
                        nc.tensor.matmul(
                            po[0 : DK + 1, qoff:512],
                            vh[h][:, kc, :],
                            pt[:, 0:nv],
                            start=(kc == 0),
                            stop=(kc == nkc - 1),
                        )
                        pop_filler()
                        if len(proj_q) > 8:
                            pop_filler()
                    nc.scalar.copy(
                        ct[h * DK : (h + 1) * DK, nb * 512 : (nb + 1) * 512],
                        po[0:DK, :],
                    )
                    nc.vector.tensor_copy(
                        qTa[h][96:97, nb * 512 : (nb + 1) * 512],
                        po[DK : DK + 1, :],
                    )
                    for j in range(4):
                        qb = nb * 4 + j
                        nc.sync.dma_start(
                            lcol[h][:, qb : qb + 1],
                            qTa[h][96:97, qb * P : (qb + 1) * P],
                        )
                    nc.vector.reciprocal(
                        rcol[h][:, nb * 4 : nb * 4 + 4],
                        lcol[h][:, nb * 4 : nb * 4 + 4],
                    )

                def wo_unit(qc, eb):
                    def emit():
                        ysb0 = ypool.tile([P, 512], F32, tag="ysb0", name="ysb0")
                        ysb1 = ypool.tile([P, 512], F32, tag="ysb1", name="ysb1")
                        psys = []
                        for h in range(NH):
                            psy = ps_proj.tile([P, 512], F32, tag="proj", name="psy")
                            nc.tensor.matmul(
                                psy[:],
                                ct[h * DK : (h + 1) * DK, qc * P : (qc + 1) * P],
                                wo_sb[h * DK : (h + 1) * DK, eb * 512 : (eb + 1) * 512],
                                start=True,
                                stop=True,
                            )
                            psys.append(psy)
                        for h, ysb in ((0, ysb0), (1, ysb1)):
                            nc.scalar.activation(
                                ysb[:], psys[h][:], ID, scale=rcol[h][:, qc : qc + 1]
                            )
                        ysbo = ypool.tile([P, 512], BF16, tag="ysbo", name="ysbo")
                        nc.gpsimd.tensor_add(out=ysbo[:], in0=ysb0[:], in1=ysb1[:])
                        nc.sync.dma_start(
                            y0[qc * P : (qc + 1) * P, eb * 512 : (eb + 1) * 512],
                            ysbo[:],
                        )

                    return emit

                def queue_wo(qcs):
                    for qc in qcs:
                        for eb in range(2):
                            wo_q.append(wo_unit(qc, eb))

                # Fully fused (default): block 0 projected directly, every
                # later K/Q projection block enters as dc-chunk fillers popped
                # between S^T kc steps (priority proj > stats > wo), so PE
                # gets proj work while ACT paces exp and DVE drains stats
                # reduces. ATTN_FUSEPROJ=0 projects everything upfront.
                fuse_proj = bool(int(os.environ.get("ATTN_FUSEPROJ", "0")))
                if fuse_proj:
                    proj_emit(1, 0)  # K block 0
                    proj_emit(0, 0)  # Q block 0
                else:
                    for nb in range(NB):
                        proj_emit(1, nb)
                    for nb in range(NB):
                        proj_emit(0, nb)
                queue_stats(0)
                drain_stats(0)
                for nb in range(NB):
                    if nb + 1 < NB:
                        if fuse_proj:
                            queue_proj(nb + 1)
                        queue_stats(nb + 1)
                    if nb >= 1:
                        queue_wo(range((nb - 1) * 4, nb * 4))
                    vproj_emit(nb)
                    drain_proj(nb)
                    drain_stats(nb)
                    st3_emit(0, nb)
                    st3_emit(1, nb)
                queue_wo(range((NB - 1) * 4, NB * 4))
                while proj_q or stats_q or wo_q:
                    pop_filler()

    _split_waits(nc)
    return nc


_cache = {}


def _get_nc(causal: bool):
    if causal not in _cache:
        nc = bass.Bass(trn_type="TRN2")
        build(nc, causal=causal)
        _cache[causal] = nc
    return _cache[causal]


def _host_masks():
    p = np.arange(P)[:, None]
    j = np.arange(P)[None, :]
    # S^T diag tile [kc, q]: nonzero only in the first 128 q-cols: p > j
    maskf = np.where(p > j, NEG, 0.0).astype(ml_dtypes.bfloat16)
    # stats diag tile [q, kc]: nonzero only in the last 128 kc-cols: j > p
    maskb = np.where(j > p, NEG, 0.0).astype(ml_dtypes.bfloat16)
    return maskf, maskb


def make_in_maps(np_inputs):
    Q = np.asarray(np_inputs["Q"], dtype=np.float32)
    K = np.asarray(np_inputs["K"], dtype=np.float32)
    V = np.asarray(np_inputs["V"], dtype=np.float32)
    W_Q = np.asarray(np_inputs["W_Q"], dtype=np.float32)
    W_K = np.asarray(np_inputs["W_K"], dtype=np.float32)
    W_V = np.asarray(np_inputs["W_V"], dtype=np.float32)
    W_O = np.asarray(np_inputs["W_O"], dtype=np.float32)

    def bf16_pair(x):
        hi = x.astype(ml_dtypes.bfloat16)
        lo = (x - hi.astype(np.float32)).astype(ml_dtypes.bfloat16)
        return np.ascontiguousarray(hi), np.ascontiguousarray(lo)

    qThl = np.ascontiguousarray(np.stack(bf16_pair(Q.T), axis=1))
    kThl = np.ascontiguousarray(np.stack(bf16_pair(K.T), axis=1))
    vTh = np.ascontiguousarray(V.T.astype(ml_dtypes.bfloat16))
    maskf, maskb = _host_masks()
    ones_row = np.ones((1, S), dtype=np.float32)

    scale = np.float32(1.0 / np.sqrt(DK))
    in_maps = []
    for c in range(NCORES):
        h0, h1 = 2 * c, 2 * c + 1
        wq2 = np.ascontiguousarray(
            np.concatenate([W_Q[h0] * scale, W_Q[h1] * scale], axis=1)
        ).astype(np.float32)
        wk2 = np.ascontiguousarray(
            np.concatenate([W_K[h0], W_K[h1]], axis=1)
        ).astype(np.float32)
        wqhi, wqlo = bf16_pair(wq2)
        wkhi, wklo = bf16_pair(wk2)
        wv2 = np.ascontiguousarray(
            np.concatenate([W_V[h0], W_V[h1]], axis=1).astype(ml_dtypes.bfloat16)
        )
        wo2 = np.ascontiguousarray(W_O[P * c : P * (c + 1), :])
        if bool(int(os.environ.get("ATTN_BF16P", "1"))):
            wo2 = wo2.astype(ml_dtypes.bfloat16)
        in_maps.append(
            {
                "qThl": qThl,
                "kThl": kThl,
                "vT": vTh,
                "wqhi": wqhi,
                "wqlo": wqlo,
                "wkhi": wkhi,
                "wklo": wklo,
                "wv": wv2,
                "wo": wo2,
                "maskf": maskf,
                "maskb": maskb,
                "onesr": ones_row,
            }
        )
    return in_maps


LAST_EXEC_NS = None


def kernel(Q, K, V, W_Q, W_K, W_V, W_O, mask):
    global LAST_EXEC_NS
    causal = bool(np.asarray(mask).item())
    nc = _get_nc(causal)
    in_maps = make_in_maps(
        dict(Q=Q, K=K, V=V, W_Q=W_Q, W_K=W_K, W_V=W_V, W_O=W_O)
    )

    trace = bool(int(os.environ.get("ATTN_TRACE", "0")))
    res = run_bass_kernel_spmd(
        nc, in_maps, core_ids=list(range(NCORES)), trace=trace
    )
    LAST_EXEC_NS = res.exec_time_ns

    out = np.zeros((S, D), dtype=np.float32)
    for c in range(NCORES):
        out += res.results[c]["y0"].astype(np.float32)
    return out


# revision 52
# speedup vs baseline: 1.5013x; 1.0393x over previous
"""Multi-head causal attention (SEQ=4096, D=1024, H=16, DK=64) on 8 TRN2
NeuronCores, tensor-parallel over heads (2 heads/core). Self-contained.

Per-core pipeline (v2):
  1. Projections: Qh^T/Kh^T/Vh^T = W.T @ X^T in fp32 (X^T pre-transposed on
     host, 1/sqrt(dk) folded into W_Q host-side). PSUM results copied to f32r
     tiles: per-head qT/kT (rows 0-63 + aux rows) and 2-head packed
     qhT2/khT2 (h0 -> partitions 0-63, h1 -> 64-127) for row-tiled stats.
  2. Stats pass: S = Qh^T.T @ Kh^T, single f32r matmul per tile, the two
     heads' matmuls row-tiled via tile_position (0,0)/(64,0) so they run
     concurrently in disjoint array row-groups. Causal mask via
     identity-matmul of a -1e9 tile. Row-max reduced on DVE, negated, and
     DMA-transposed into row 64 of the per-head qT tile.
  3. S^T pass: single f32r matmul S^T[kc,q] = [Kh;1].T @ [Qh;-m] (the max
     subtraction rides the contraction as the 65th row). f32r operand
     rounding gives score error ~5e-2 abs (on a +-3500 range), which maps to
     ~2e-3 output rel err (empirically calibrated) - far inside the 2e-2
     gate. Mask via identity matmul, ACT exp -> P^T. AV in f32r:
     ones-augmented Vh gives l = sum(exp) as row 64 of the PSUM accumulator.
  4. 1/l per qb (DVE reciprocal), Y_partial = C^T.T @ W_O_rows with the two
     heads' matmuls row-tiled, scaled by 1/l on ACT, summed on DVE, DMA out.
     Host sums the 8 per-core partials.
"""

import os
import sys

sys.path.insert(0, "/opt/trn_rl_repo")

import numpy as np
import ml_dtypes

import concourse.bass as bass
import concourse.mybir as mybir
import concourse.tile as tile
from concourse.bass_utils import run_bass_kernel_spmd
from concourse.masks import make_identity

P = 128
S = 4096
D = 1024
DK = 64
NH = 2  # heads per core
NCORES = 8
NEG = -1.0e9
F32 = mybir.dt.float32
F32R = mybir.dt.float32r
BF16 = mybir.dt.bfloat16
EXP = mybir.ActivationFunctionType.Exp

_ctr = [0]


def _split_waits(nc, max_waits=1):
    """walrus rejects >1 sem-wait per instruction; move extras onto
    preceding same-engine NOPs (engine streams are program-ordered)."""
    for f in nc.m.functions:
        for bb in f.blocks:
            insts = bb.instructions
            new = []
            changed = False
            for inst in insts:
                si = inst.sync_info
                if si is not None and si.on_wait and len(si.on_wait) > max_waits:
                    waits = list(si.on_wait)
                    extra, keep = waits[:-max_waits], waits[-max_waits:]
                    for i in range(0, len(extra), max_waits):
                        _ctr[0] += 1
                        new.append(
                            mybir.InstNoOp(
                                name=f"waitsplit-{_ctr[0]}",
                                engine=inst.engine,
                                ins=[],
                                outs=[],
                                sync_info=mybir.SyncInfo(
                                    on_wait=extra[i : i + max_waits], on_update=[]
                                ),
                            )
                        )
                    inst.sync_info = mybir.SyncInfo(
                        on_wait=keep, on_update=list(si.on_update)
                    )
                    changed = True
                new.append(inst)
            if changed:
                bb.instructions = new


def build(nc: bass.Bass, causal: bool = True):
    repeat = int(os.environ.get("ATTN_REPEAT", "1"))
    # bf16 for the max-stats operands, P/V/C/W_O: bf16 matmuls get a
    # standalone pipelined LDWEIGHTS (f32r must self-load serially on HW)
    bf16p = bool(int(os.environ.get("ATTN_BF16P", "1")))
    DT_P = BF16 if bf16p else F32R
    NB = S // 512  # 8   512-wide blocks
    QB = S // P  # 32  128-wide q blocks
    DC = D // P  # 8   128-deep contraction chunks

    qThl = nc.dram_tensor("qThl", [D, 2, S], BF16, kind="ExternalInput")
    kThl = nc.dram_tensor("kThl", [D, 2, S], BF16, kind="ExternalInput")
    vT = nc.dram_tensor("vT", [D, S], BF16, kind="ExternalInput")
    wqhi = nc.dram_tensor("wqhi", [D, NH * DK], BF16, kind="ExternalInput")
    wqlo = nc.dram_tensor("wqlo", [D, NH * DK], BF16, kind="ExternalInput")
    wkhi = nc.dram_tensor("wkhi", [D, NH * DK], BF16, kind="ExternalInput")
    wklo = nc.dram_tensor("wklo", [D, NH * DK], BF16, kind="ExternalInput")
    wv = nc.dram_tensor("wv", [D, NH * DK], BF16, kind="ExternalInput")
    wo = nc.dram_tensor("wo", [NH * DK, D], DT_P, kind="ExternalInput")
    maskf = nc.dram_tensor("maskf", [P, P], BF16, kind="ExternalInput")
    maskb = nc.dram_tensor("maskb", [P, P], BF16, kind="ExternalInput")
    onesr = nc.dram_tensor("onesr", [1, S], F32R, kind="ExternalInput")
    y0 = nc.dram_tensor("y0", [S, D], BF16, kind="ExternalOutput")

    with tile.TileContext(nc) as tc:
        import contextlib

        ctx = contextlib.ExitStack()
        with ctx:
            const = ctx.enter_context(tc.tile_pool(name="const", bufs=1))
            big = ctx.enter_context(tc.tile_pool(name="big", bufs=1))
            stream = ctx.enter_context(
                tc.tile_pool(name="stream", bufs=int(os.environ.get("ATTN_BSTREAM", "6")))
            )
            pstream = ctx.enter_context(
                tc.tile_pool(name="pstream", bufs=int(os.environ.get("ATTN_BPS", "16")))
            )
            ptp = ctx.enter_context(
                tc.tile_pool(name="ptp", bufs=int(os.environ.get("ATTN_BPT", "3")))
            )
            ypool = ctx.enter_context(
                tc.tile_pool(name="ypool", bufs=int(os.environ.get("ATTN_BY", "2")))
            )
            smalls = ctx.enter_context(
                tc.tile_pool(name="smalls", bufs=int(os.environ.get("ATTN_BSM", "4")))
            )
            bproj = int(os.environ.get("ATTN_BPROJ", "2"))
            bstat = int(os.environ.get("ATTN_BSTAT", "2"))
            bst = int(os.environ.get("ATTN_BST", "2"))
            bmisc = int(os.environ.get("ATTN_BMISC", "2"))
            ps_proj = ctx.enter_context(
                tc.tile_pool(name="ps_proj", bufs=bproj, space="PSUM")
            )
            ps_stat = ctx.enter_context(
                tc.tile_pool(name="ps_stat", bufs=bstat, space="PSUM")
            )
            ps_st = ctx.enter_context(tc.tile_pool(name="ps_st", bufs=bst, space="PSUM"))
            ps_misc = ctx.enter_context(
                tc.tile_pool(name="ps_misc", bufs=bmisc, space="PSUM")
            )

            # ---- constants ----
            ident = const.tile([P, P], F32)
            make_identity(nc, ident[:])
            ident_b = const.tile([P, P], BF16)
            nc.vector.tensor_copy(ident_b[:], ident[:])
            ident_r = const.tile([P, P], F32R)
            nc.vector.tensor_copy(ident_r[:], ident[:])

            wq_hi_sb = const.tile([P, DC, P], BF16, tag="wqh")
            wq_lo_sb = const.tile([P, DC, P], BF16, tag="wql")
            wk_hi_sb = const.tile([P, DC, P], BF16, tag="wkh")
            wk_lo_sb = const.tile([P, DC, P], BF16, tag="wkl")
            wv_sb = const.tile([P, DC, P], BF16, tag="wv")
            nc.sync.dma_start(wq_hi_sb[:], wqhi.rearrange("(o p) m -> p o m", p=P))
            nc.sync.dma_start(wq_lo_sb[:], wqlo.rearrange("(o p) m -> p o m", p=P))
            nc.sync.dma_start(wk_hi_sb[:], wkhi.rearrange("(o p) m -> p o m", p=P))
            nc.sync.dma_start(wk_lo_sb[:], wklo.rearrange("(o p) m -> p o m", p=P))
            nc.sync.dma_start(wv_sb[:], wv.rearrange("(o p) m -> p o m", p=P))
            wo_sb = const.tile([P, D], DT_P, tag="wo")
            nc.sync.dma_start(wo_sb[:], wo[:])

            mf_sb = const.tile([P, P], BF16, tag="mf")
            mb_sb = const.tile([P, P], BF16, tag="mb")
            nc.sync.dma_start(mf_sb[:], maskf[:])
            nc.sync.dma_start(mb_sb[:], maskb[:])

            # ---- persistent activations ----
            # qTa[h]: rows 0-63 = Qh^T (f32r), row 64 = -m (written per qb by
            # stats), row 96 = l stash. kTa[h]: rows 0-63 = Kh^T, row 64 = ones.
            # qhT2/khT2: 2-head packed (h0 -> 0-63, h1 -> 64-127) for the
            # row-tiled stats matmuls.
            qTa = [big.tile([P, S], F32R, tag=f"qTa{h}", name=f"qTa{h}") for h in range(NH)]
            kTa = [big.tile([P, S], F32R, tag=f"kTa{h}", name=f"kTa{h}") for h in range(NH)]
            qhT2 = big.tile([P, S], DT_P, tag="qhT2", name="qhT2")
            khT2 = big.tile([P, S], DT_P, tag="khT2", name="khT2")
            vh = [big.tile([P, QB, DK + 1], DT_P, tag=f"vh{h}", name=f"vh{h}") for h in range(NH)]
            ct = big.tile([P, S], DT_P, tag="ct")
            mcol = [big.tile([P, QB], F32R, tag=f"mcol{h}", name=f"mcol{h}") for h in range(NH)]
            lcol = [big.tile([P, QB], F32R, tag=f"lcol{h}", name=f"lcol{h}") for h in range(NH)]
            rcol = [big.tile([P, QB], F32, tag=f"rcol{h}", name=f"rcol{h}") for h in range(NH)]

            ones_qb = const.tile([P, QB], F32, tag="ones_qb")
            nc.any.memset(ones_qb[:], 1.0)
            for h in range(NH):
                nc.sync.dma_start(kTa[h][DK : DK + 1, :], onesr[:])  # ones row
                nc.vector.tensor_copy(vh[h][:, :, DK], ones_qb[:])  # ones col

            for _rep in range(repeat):
                # ---- Q/K projection block emitters ----
                # fp32 accuracy via bf16 hi/lo pairs (host-split): X@W =
                # Xhi@Whi + Xhi@Wlo + Xlo@Whi (lo*lo term negligible), each
                # a 1-cycle/row bf16 matmul vs fp32's 4 cycles/row.
                def proj_emit(t_idx, nb):
                    xhl, w_hi, w_lo = [
                        (qThl, wq_hi_sb, wq_lo_sb),
                        (kThl, wk_hi_sb, wk_lo_sb),
                    ][t_idx]
                    ps = ps_proj.tile([P, 512], F32, tag="proj", name="ps")
                    for dc in range(DC):
                        xt = stream.tile([P, 2, 512], BF16, tag="xin", name="xt")
                        dsl = slice(dc * P, (dc + 1) * P)
                        nsl = slice(nb * 512, (nb + 1) * 512)
                        nc.sync.dma_start(xt[:], xhl[dsl, :, nsl])
                        nc.tensor.matmul(
                            ps[:], w_hi[:, dc, :], xt[:, 0, :],
                            start=(dc == 0), stop=False,
                        )
                        nc.tensor.matmul(
                            ps[:], w_lo[:, dc, :], xt[:, 0, :],
                            start=False, stop=False,
                        )
                        nc.tensor.matmul(
                            ps[:], w_hi[:, dc, :], xt[:, 1, :],
                            start=False, stop=(dc == DC - 1),
                        )
                    sl = slice(nb * 512, (nb + 1) * 512)
                    pk2 = qhT2 if t_idx == 0 else khT2
                    dst = qTa if t_idx == 0 else kTa
                    nc.scalar.copy(pk2[:, sl], ps[:])
                    for h in range(NH):
                        nc.scalar.copy(
                            dst[h][0:DK, sl], ps[h * DK : (h + 1) * DK, :]
                        )

                def vproj_emit(nb):
                    ps = ps_proj.tile([P, 512], F32, tag="proj", name="ps")
                    for dc in range(DC):
                        xt = stream.tile([P, 512], BF16, tag="xin", name="xtv")
                        nc.sync.dma_start(
                            xt[:],
                            vT[dc * P : (dc + 1) * P, nb * 512 : (nb + 1) * 512],
                        )
                        nc.tensor.matmul(
                            ps[:],
                            wv_sb[:, dc, :],
                            xt[:],
                            start=(dc == 0),
                            stop=(dc == DC - 1),
                        )
                    vtmp = stream.tile([P, 512], F32R, tag="xin", name="vtmp")
                    nc.vector.tensor_copy(vtmp[:], ps[:])
                    for h in range(NH):
                        pst = ps_misc.tile([P, 512], F32R, tag="misc", name="pst")
                        for j in range(4):
                            nc.tensor.transpose(
                                pst[0:P, j * DK : (j + 1) * DK],
                                vtmp[h * DK : (h + 1) * DK, j * P : (j + 1) * P],
                                ident_r[h * DK : (h + 1) * DK, h * DK : (h + 1) * DK],
                            )
                        nc.vector.tensor_copy(
                            vh[h][:, nb * 4 : nb * 4 + 4, 0:DK],
                            pst[:, 0 : 4 * DK].rearrange("p (j d) -> p j d", j=4),
                        )

                # ---- stats: row max via row-tiled 2-head matmuls ----
                # Emitted as fine-grained filler units popped between S^T kc
                # steps so the DVE reduces drain under S^T compute and the
                # two ps_stat banks recycle without stalling PE.
                ID = mybir.ActivationFunctionType.Identity
                from collections import deque

                stats_q = deque()  # (batch_nb, emit)
                wo_q = deque()
                proj_q = deque()  # (block_nb, emit) dc-chunk fillers
                mparts_of = {}
                acc_of = {}

                def proj_unit(t_idx, nb, dc, xt):
                    def emit():
                        key = (t_idx, nb)
                        w_hi, w_lo = [
                            (wq_hi_sb, wq_lo_sb),
                            (wk_hi_sb, wk_lo_sb),
                        ][t_idx]
                        if dc == 0:
                            acc_of[key] = ps_proj.tile(
                                [P, 512], F32, tag="proj", name="ps"
                            )
                        ps = acc_of[key]
                        nc.tensor.matmul(
                            ps[:], w_hi[:, dc, :], xt[:, 0, :],
                            start=(dc == 0), stop=False,
                        )
                        nc.tensor.matmul(
                            ps[:], w_lo[:, dc, :], xt[:, 0, :],
                            start=False, stop=False,
                        )
                        nc.tensor.matmul(
                            ps[:], w_hi[:, dc, :], xt[:, 1, :],
                            start=False, stop=(dc == DC - 1),
                        )
                        if dc == DC - 1:
                            del acc_of[key]
                            sl = slice(nb * 512, (nb + 1) * 512)
                            pk2 = qhT2 if t_idx == 0 else khT2
                            dst = qTa if t_idx == 0 else kTa
                            nc.scalar.copy(pk2[:, sl], ps[:])
                            for h in range(NH):
                                nc.scalar.copy(
                                    dst[h][0:DK, sl],
                                    ps[h * DK : (h + 1) * DK, :],
                                )

                    return emit

                def queue_proj(nb):
                    # DMAs are issued at queue time (one iteration ahead of
                    # consumption) so their latency hides under the S^T steps
                    for t_idx in (1, 0):  # K then Q
                        xhl = [qThl, kThl][t_idx]
                        for dc in range(DC):
                            xt = pstream.tile([P, 2, 512], BF16, tag="pxin", name="xt")
                            nc.sync.dma_start(
                                xt[:],
                                xhl[dc * P : (dc + 1) * P, :, nb * 512 : (nb + 1) * 512],
                            )
                            proj_q.append((nb, proj_unit(t_idx, nb, dc, xt)))

                def drain_proj(max_nb):
                    while proj_q and proj_q[0][0] <= max_nb:
                        proj_q.popleft()[1]()

                def pop_filler():
                    # proj before stats: stats units read the proj copies, so
                    # their matmuls must trail them in the PE stream
                    if proj_q:
                        proj_q.popleft()[1]()
                        return True
                    if stats_q:
                        stats_q.popleft()[1]()
                        return True
                    if wo_q:
                        wo_q.popleft()()
                        return True
                    return False

                def stats_unit(qb, kc):
                    def emit():
                        kmax = qb // 4 + 1 if causal else NB
                        if kc == 0:
                            mparts_of[qb] = [
                                smalls.tile([P, NB], F32, tag="mpart", name=f"mpart{h}")
                                for h in range(NH)
                            ]
                        mparts = mparts_of[qb]
                        diag = causal and (kc == qb // 4)
                        nv = (qb % 4) * P + P if diag else 512
                        pss = []
                        for h in range(NH):
                            ps = ps_stat.tile([P, 512], F32, tag="stat", name="ps_stat")
                            nc.tensor.matmul(
                                ps[:, 0:nv],
                                qhT2[h * DK : (h + 1) * DK, qb * P : (qb + 1) * P],
                                khT2[h * DK : (h + 1) * DK, kc * 512 : kc * 512 + nv],
                                start=True,
                                stop=not diag,
                            )
                            pss.append(ps)
                        for h in range(NH):
                            if diag:
                                nc.tensor.matmul(
                                    pss[h][:, nv - P : nv],
                                    ident_b[:],
                                    mb_sb[:],
                                    start=False,
                                    stop=True,
                                )
                            nc.vector.reduce_max(
                                mparts[h][:, kc : kc + 1],
                                pss[h][:, 0:nv],
                                axis=mybir.AxisListType.X,
                            )
                        if kc == kmax - 1:
                            del mparts_of[qb]
                            for h in range(NH):
                                nc.vector.tensor_reduce(
                                    mcol[h][:, qb : qb + 1],
                                    mparts[h][:, 0:kmax],
                                    axis=mybir.AxisListType.X,
                                    op=mybir.AluOpType.max,
                                    negate=True,
                                )
                                nc.sync.dma_start(
                                    qTa[h][DK : DK + 1, qb * P : (qb + 1) * P],
                                    mcol[h][:, qb : qb + 1],
                                )

                    return emit

                def queue_stats(batch):
                    for qb in range(4 * batch, 4 * batch + 4):
                        kmax = qb // 4 + 1 if causal else NB
                        for kc in range(kmax):
                            stats_q.append((batch, stats_unit(qb, kc)))

                def drain_stats(max_batch):
                    # force-complete stats units for batches <= max_batch
                    # (gates the S^T pass that reads their m rows)
                    while stats_q and stats_q[0][0] <= max_batch:
                        stats_q.popleft()[1]()

                # ---- S^T + exp + AV ----
                def st3_emit(h, nb):
                    nkc = 4 * (nb + 1) if causal else QB
                    po = ps_misc.tile([P, 512], F32, tag="misc", name="po")
                    pss = {}

                    def s_mm(kc):
                        ps = ps_st.tile([P, 512], F32, tag="st", name="ps_st")
                        diag = causal and (kc >= 4 * nb)
                        o = kc - 4 * nb if diag else 0
                        qoff = o * P
                        nv = 512 - qoff
                        kslice = slice(kc * P, (kc + 1) * P)
                        qslice = slice(nb * 512 + qoff, (nb + 1) * 512)
                        nc.tensor.matmul(
                            ps[:, 0:nv],
                            kTa[h][0 : DK + 1, kslice],
                            qTa[h][0 : DK + 1, qslice],
                            start=True,
                            stop=not diag,
                        )
                        if diag:
                            nc.tensor.matmul(
                                ps[:, 0:P],
                                ident_b[:],
                                mf_sb[:],
                                start=False,
                                stop=True,
                            )
                        pss[kc] = (ps, qoff, nv)

                    s_mm(0)
                    for kc in range(nkc):
                        if kc + 1 < nkc:
                            s_mm(kc + 1)
                        ps, qoff, nv = pss.pop(kc)
                        pt = ptp.tile([P, 512], DT_P, tag="pt", name="pt")
                        nc.scalar.activation(pt[:, 0:nv], ps[:, 0:nv], EXP)
                        nc.tensor.matmul(
                            po[0 : DK + 1, qoff:512],
                            vh[h][:, kc, :],
                            pt[:, 0:nv],
                            start=(kc == 0),
                            stop=(kc == nkc - 1),
                        )
                        pop_filler()
                        if len(proj_q) > 8:
                            pop_filler()
                    nc.scalar.copy(
                        ct[h * DK : (h + 1) * DK, nb * 512 : (nb + 1) * 512],
                        po[0:DK, :],
                    )
                    nc.vector.tensor_copy(
                        qTa[h][96:97, nb * 512 : (nb + 1) * 512],
                        po[DK : DK + 1, :],
                    )
                    for j in range(4):
                        qb = nb * 4 + j
                        nc.sync.dma_start(
                            lcol[h][:, qb : qb + 1],
                            qTa[h][96:97, qb * P : (qb + 1) * P],
                        )
                    nc.vector.reciprocal(
                        rcol[h][:, nb * 4 : nb * 4 + 4],
                        lcol[h][:, nb * 4 : nb * 4 + 4],
                    )

                def wo_unit(qc, eb):
                    def emit():
                        ysb0 = ypool.tile([P, 512], F32, tag="ysb0", name="ysb0")
                        ysb1 = ypool.tile([P, 512], F32, tag="ysb1", name="ysb1")
                        psys = []
                        for h in range(NH):
                            psy = ps_proj.tile([P, 512], F32, tag="proj", name="psy")
                            nc.tensor.matmul(
                                psy[:],
                                ct[h * DK : (h + 1) * DK, qc * P : (qc + 1) * P],
                                wo_sb[h * DK : (h + 1) * DK, eb * 512 : (eb + 1) * 512],
                                start=True,
                                stop=True,
                            )
                            psys.append(psy)
                        for h, ysb in ((0, ysb0), (1, ysb1)):
                            nc.scalar.activation(
                                ysb[:], psys[h][:], ID, scale=rcol[h][:, qc : qc + 1]
                            )
                        ysbo = ypool.tile([P, 512], BF16, tag="ysbo", name="ysbo")
                        nc.gpsimd.tensor_add(out=ysbo[:], in0=ysb0[:], in1=ysb1[:])
                        nc.sync.dma_start(
                            y0[qc * P : (qc + 1) * P, eb * 512 : (eb + 1) * 512],
                            ysbo[:],
                        )

                    return emit

                def queue_wo(qcs):
                    for qc in qcs:
                        for eb in range(2):
                            wo_q.append(wo_unit(qc, eb))

                # Fully fused (default): block 0 projected directly, every
                # later K/Q projection block enters as dc-chunk fillers popped
                # between S^T kc steps (priority proj > stats > wo), so PE
                # gets proj work while ACT paces exp and DVE drains stats
                # reduces. ATTN_FUSEPROJ=0 projects everything upfront.
                fuse_proj = bool(int(os.environ.get("ATTN_FUSEPROJ", "0")))
                if fuse_proj:
                    proj_emit(1, 0)  # K block 0
                    proj_emit(0, 0)  # Q block 0
                else:
                    for nb in range(NB):
                        proj_emit(1, nb)
                    for nb in range(NB):
                        proj_emit(0, nb)
                queue_stats(0)
                drain_stats(0)
                for nb in range(NB):
                    if nb + 1 < NB:
                        if fuse_proj:
                            queue_proj(nb + 1)
                        queue_stats(nb + 1)
                    if nb >= 1:
                        queue_wo(range((nb - 1) * 4, nb * 4))
                    vproj_emit(nb)
                    drain_proj(nb)
                    drain_stats(nb)
                    st3_emit(0, nb)
                    st3_emit(1, nb)
                queue_wo(range((NB - 1) * 4, NB * 4))
                while proj_q or stats_q or wo_q:
                    pop_filler()

    _split_waits(nc)
    return nc


_cache = {}


def _get_nc(causal: bool):
    if causal not in _cache:
        nc = bass.Bass(trn_type="TRN2")
        build(nc, causal=causal)
        _cache[causal] = nc
    return _cache[causal]


def _host_masks():
    p = np.arange(P)[:, None]
    j = np.arange(P)[None, :]
    # S^T diag tile [kc, q]: nonzero only in the first 128 q-cols: p > j
    maskf = np.where(p > j, NEG, 0.0).astype(ml_dtypes.bfloat16)
    # stats diag tile [q, kc]: nonzero only in the last 128 kc-cols: j > p
    maskb = np.where(j > p, NEG, 0.0).astype(ml_dtypes.bfloat16)
    return maskf, maskb


def make_in_maps(np_inputs):
    Q = np.asarray(np_inputs["Q"], dtype=np.float32)
    K = np.asarray(np_inputs["K"], dtype=np.float32)
    V = np.asarray(np_inputs["V"], dtype=np.float32)
    W_Q = np.asarray(np_inputs["W_Q"], dtype=np.float32)
    W_K = np.asarray(np_inputs["W_K"], dtype=np.float32)
    W_V = np.asarray(np_inputs["W_V"], dtype=np.float32)
    W_O = np.asarray(np_inputs["W_O"], dtype=np.float32)

    def bf16_pair(x):
        hi = x.astype(ml_dtypes.bfloat16)
        lo = (x - hi.astype(np.float32)).astype(ml_dtypes.bfloat16)
        return np.ascontiguousarray(hi), np.ascontiguousarray(lo)

    qThl = np.ascontiguousarray(np.stack(bf16_pair(Q.T), axis=1))
    kThl = np.ascontiguousarray(np.stack(bf16_pair(K.T), axis=1))
    vTh = np.ascontiguousarray(V.T.astype(ml_dtypes.bfloat16))
    maskf, maskb = _host_masks()
    ones_row = np.ones((1, S), dtype=np.float32)

    scale = np.float32(1.0 / np.sqrt(DK))
    in_maps = []
    for c in range(NCORES):
        h0, h1 = 2 * c, 2 * c + 1
        wq2 = np.ascontiguousarray(
            np.concatenate([W_Q[h0] * scale, W_Q[h1] * scale], axis=1)
        ).astype(np.float32)
        wk2 = np.ascontiguousarray(
            np.concatenate([W_K[h0], W_K[h1]], axis=1)
        ).astype(np.float32)
        wqhi, wqlo = bf16_pair(wq2)
        wkhi, wklo = bf16_pair(wk2)
        wv2 = np.ascontiguousarray(
            np.concatenate([W_V[h0], W_V[h1]], axis=1).astype(ml_dtypes.bfloat16)
        )
        wo2 = np.ascontiguousarray(W_O[P * c : P * (c + 1), :])
        if bool(int(os.environ.get("ATTN_BF16P", "1"))):
            wo2 = wo2.astype(ml_dtypes.bfloat16)
        in_maps.append(
            {
                "qThl": qThl,
                "kThl": kThl,
                "vT": vTh,
                "wqhi": wqhi,
                "wqlo": wqlo,
                "wkhi": wkhi,
                "wklo": wklo,
                "wv": wv2,
                "wo": wo2,
                "maskf": maskf,
                "maskb": maskb,
                "onesr": ones_row,
            }
        )
    return in_maps


LAST_EXEC_NS = None


def kernel(Q, K, V, W_Q, W_K, W_V, W_O, mask):
    global LAST_EXEC_NS
    causal = bool(np.asarray(mask).item())
    nc = _get_nc(causal)
    in_maps = make_in_maps(
        dict(Q=Q, K=K, V=V, W_Q=W_Q, W_K=W_K, W_V=W_V, W_O=W_O)
    )

    trace = bool(int(os.environ.get("ATTN_TRACE", "0")))
    res = run_bass_kernel_spmd(
        nc, in_maps, core_ids=list(range(NCORES)), trace=trace
    )
    LAST_EXEC_NS = res.exec_time_ns

    out = np.zeros((S, D), dtype=np.float32)
    for c in range(NCORES):
        out += res.results[c]["y0"].astype(np.float32)
    return out
